# revision 1
# baseline (speedup 1.0000x reference)
"""Trainium2 Bass kernel for a dense pre-norm transformer block (B=2, T=2048,
D=2048, H=16, DH=128, FG=5461, SwiGLU MLP, RoPE, causal attention).

Sharding: tensor-parallel attention over heads (2 heads/core on 8 cores), one
AllToAll to reshard to 512 query columns per core, then fully local
proj + MLP per core (weights replicated, streamed from HBM).

All on-device activations are kept transposed ([feature, row]) so every
matmul is lhsT(=weight tile).T @ rhs(=activationT tile) with the contraction
dim on SBUF partitions. Matmuls run in float32r.
"""

import numpy as np

import concourse.bass as bass
import concourse.mybir as mybir
import concourse.tile as tile
from concourse import bacc
from concourse.bass_utils import run_bass_kernel_spmd

# Problem constants
B, T, D = 2, 2048, 2048
H, DH = 16, 128
FG = 5461
EPS = 1e-5
ROPE_BASE = 10000.0

P = 128
NCORES = 8
R = B * T                    # 4096 rows total
RB = 512                     # rows per block / per-core q-cols
NRB = R // RB                # 8 row blocks
DT = D // P                  # 16 d-tiles
HPC = H // NCORES            # 2 heads per core
NF = 3 * HPC                 # 6 feature tiles per core in qkv (q0,q1,k0,k1,v0,v1)
FGP = 5504                   # FG padded to 43*128
FGT = FGP // P               # 43 fg tiles
KTB = T // P                 # 16 k-tiles per batch
QBB = T // RB                # 4 q-blocks per batch
NEG = -1.0e30
SCALE = 1.0 / np.sqrt(DH)

SIM_SILU = False  # replace Silu with sigmoid+mults (for CoreSim, which lacks Silu)

F32 = mybir.dt.float32
F32R = mybir.dt.float32r
EXP = mybir.ActivationFunctionType.Exp
SQUARE = mybir.ActivationFunctionType.Square
SILU = mybir.ActivationFunctionType.Silu
MULT = mybir.AluOpType.mult
ADD = mybir.AluOpType.add


def _build_program():
    nc = bacc.Bacc("TRN2", target_bir_lowering=False, debug=False, num_devices=NCORES)

    # ---- per-core external inputs ----
    xT = nc.dram_tensor("xT", [D, R], F32, kind="ExternalInput")
    xres = nc.dram_tensor("xres", [DT, P, RB], F32, kind="ExternalInput")
    wqkv = nc.dram_tensor("wqkv", [P, NF, DT, P], F32, kind="ExternalInput")
    wproj = nc.dram_tensor("wproj", [P, DT, DT, P], F32, kind="ExternalInput")
    wgate = nc.dram_tensor("wgate", [P, FGT, DT, P], F32, kind="ExternalInput")
    wval = nc.dram_tensor("wval", [P, FGT, DT, P], F32, kind="ExternalInput")
    wmlp = nc.dram_tensor("wmlp", [P, DT, FGT, P], F32, kind="ExternalInput")
    cosT = nc.dram_tensor("cosT", [P, T], F32, kind="ExternalInput")
    sinT = nc.dram_tensor("sinT", [P, T], F32, kind="ExternalInput")
    rotPT = nc.dram_tensor("rotPT", [P, P], F32, kind="ExternalInput")
    masks = nc.dram_tensor("masks", [P, QBB, RB], F32, kind="ExternalInput")
    ones_in = nc.dram_tensor("ones_in", [P, P], F32, kind="ExternalInput")
    ident_in = nc.dram_tensor("ident_in", [P, P], F32, kind="ExternalInput")

    outT = nc.dram_tensor("outT", [DT, P, RB], F32, kind="ExternalOutput")

    # ---- internal DRAM scratch ----
    qkv_dram = nc.dram_tensor("qkv_dram", [NF, P, R], F32)
    a2a_in = [nc.dram_tensor(f"a2a_in{h}", [NCORES, P, RB], F32) for h in range(HPC)]
    a2a_out = [nc.dram_tensor(f"a2a_out{h}", [NCORES, P, RB], F32) for h in range(HPC)]
    x2_dram = nc.dram_tensor("x2_dram", [DT, P, RB], F32)

    with tile.TileContext(nc) as tc:
        with tc.tile_pool(name="const", bufs=1) as cpool:
            cos_t = cpool.tile([P, T], F32)
            sin_t = cpool.tile([P, T], F32)
            rot_t = cpool.tile([P, P], F32R)
            mask_t = cpool.tile([P, QBB, RB], F32)
            ones_t = cpool.tile([P, P], F32R)
            id_t = cpool.tile([P, P], F32R)
            nc.sync.dma_start(cos_t[:], cosT[:, :])
            nc.sync.dma_start(sin_t[:], sinT[:, :])
            nc.sync.dma_start(rot_t[:], rotPT[:, :].bitcast(F32R))
            nc.sync.dma_start(mask_t[:], masks[:, :, :])
            nc.sync.dma_start(ones_t[:], ones_in[:, :].bitcast(F32R))
            nc.sync.dma_start(id_t[:], ident_in[:, :].bitcast(F32R))

            # ============ Phase 1: qkv on raw x + rope, rms applied at output ==
            with (
                tc.tile_pool(name="p1w", bufs=1) as p1w,
                tc.tile_pool(name="p1x", bufs=2) as p1x,
                tc.tile_pool(name="p1t", bufs=2) as p1t,
                tc.tile_pool(name="p1ps_ss", bufs=2, space="PSUM") as p1ps_ss,
                tc.tile_pool(name="p1ps_mm", bufs=3, space="PSUM") as p1ps_mm,
                tc.tile_pool(name="p1ps_rot", bufs=2, space="PSUM") as p1ps_rot,
            ):
                wq_t = [p1w.tile([P, DT, P], F32R, tag=f"wq{f}", name=f"wq{f}")
                        for f in range(NF)]
                for f in range(NF):
                    nc.sync.dma_start(wq_t[f][:], wqkv[:, f, :, :].bitcast(F32R))
                xTr = xT.rearrange("(dt p) r -> p dt r", p=P)

                for rb in range(NRB):
                    t0 = (rb % QBB) * RB
                    xb = p1x.tile([P, DT, RB], F32R, tag="xblk")
                    for dt in range(DT):
                        nc.sync.dma_start(
                            xb[:, dt],
                            xTr[:, dt, rb * RB : (rb + 1) * RB].bitcast(F32R),
                        )
                    # rms chain (independent of qkv matmuls; scale at output)
                    sacc = p1t.tile([P, RB], F32R, tag="sacc")
                    first_sq = None
                    for dt in range(DT):
                        sq = p1t.tile([P, RB], F32R, tag="sq")
                        nc.scalar.activation(sq[:], xb[:, dt], SQUARE)
                        if dt == 0:
                            first_sq = sq
                        elif dt == 1:
                            nc.vector.tensor_tensor(sacc[:], first_sq[:], sq[:], ADD)
                        else:
                            nc.vector.tensor_tensor(sacc[:], sacc[:], sq[:], ADD)
                    ss_ps = p1ps_ss.tile([P, RB], F32, tag="ss")
                    nc.tensor.matmul(ss_ps[:], ones_t[:], sacc[:], start=True, stop=True)
                    rms = p1t.tile([P, RB], F32, tag="rms")
                    nc.vector.tensor_scalar(rms[:], ss_ps[:], 1.0 / D, EPS, MULT, ADD)
                    nc.scalar.sqrt(rms[:], rms[:])
                    nc.vector.reciprocal(rms[:], rms[:])
                    # qkv matmuls on RAW x; f: 0,1=q; 2,3=k; 4,5=v
                    for f in range(NF):
                        ps = p1ps_mm.tile([P, RB], F32, tag="qkvps")
                        for dt in range(DT):
                            nc.tensor.matmul(
                                ps[:], wq_t[f][:, dt], xb[:, dt],
                                start=(dt == 0), stop=(dt == DT - 1),
                            )
                        out_t = p1t.tile([P, RB], F32R, tag="outt")
                        if f < 2 * HPC:
                            raw = p1t.tile([P, RB], F32R, tag="raw")
                            nc.vector.tensor_copy(raw[:], ps[:])
                            rps = p1ps_rot.tile([P, RB], F32, tag="rotps")
                            nc.tensor.matmul(rps[:], rot_t[:], raw[:], start=True, stop=True)
                            m1 = p1t.tile([P, RB], F32, tag="m1")
                            nc.vector.tensor_tensor(
                                m1[:], raw[:], cos_t[:, t0 : t0 + RB], MULT)
                            m2 = p1t.tile([P, RB], F32, tag="m2")
                            nc.vector.tensor_tensor(
                                m2[:], rps[:], sin_t[:, t0 : t0 + RB], MULT)
                            rr = p1t.tile([P, RB], F32, tag="rr")
                            nc.vector.tensor_tensor(rr[:], m1[:], m2[:], ADD)
                            nc.vector.tensor_tensor(out_t[:], rr[:], rms[:], MULT)
                        else:
                            nc.vector.tensor_tensor(out_t[:], ps[:], rms[:], MULT)
                        nc.sync.dma_start(
                            qkv_dram[f, :, rb * RB : (rb + 1) * RB].bitcast(F32R),
                            out_t[:],
                        )

            # ============ Phase 2: attention, h outer (A2A per head) ============
            with (
                tc.tile_pool(name="p2kv", bufs=2) as p2kv,
                tc.tile_pool(name="p2a", bufs=2) as p2a,
                tc.tile_pool(name="p2t", bufs=3) as p2t,
                tc.tile_pool(name="p2ps_tp", bufs=2, space="PSUM") as p2ps_tp,
                tc.tile_pool(name="p2ps_s", bufs=2, space="PSUM") as p2ps_s,
                tc.tile_pool(name="p2ps_o", bufs=2, space="PSUM") as p2ps_o,
                tc.tile_pool(name="p2ps_l", bufs=2, space="PSUM") as p2ps_l,
            ):
                pairs = [(h, b) for h in range(HPC) for b in range(B)]

                def load_kv(h, b):
                    kT = p2kv.tile([P, T], F32R, tag="kT", name=f"kT{h}{b}")
                    vT = p2kv.tile([P, T], F32R, tag="vT", name=f"vT{h}{b}")
                    nc.sync.dma_start(
                        kT[:], qkv_dram[2 + h, :, b * T : (b + 1) * T].bitcast(F32R))
                    nc.sync.dma_start(
                        vT[:], qkv_dram[4 + h, :, b * T : (b + 1) * T].bitcast(F32R))
                    return kT, vT

                def transpose_v(vT, h, b):
                    v_rm = p2kv.tile([P, KTB, P], F32R, tag="v_rm", name=f"vrm{h}{b}")
                    for kt in range(KTB):
                        tps = p2ps_tp.tile([P, P], F32R, tag="vtp")
                        nc.tensor.transpose(tps[:], vT[:, kt * P : (kt + 1) * P], id_t[:])
                        nc.vector.tensor_copy(v_rm[:, kt], tps[:])
                    return v_rm

                kv = load_kv(*pairs[0])
                for pi, (h, b) in enumerate(pairs):
                    kT, vT = kv
                    v_rm = transpose_v(vT, h, b)
                    kv_next = load_kv(*pairs[pi + 1]) if pi + 1 < len(pairs) else None
                    for qb in range(QBB):
                        qTs = p2t.tile([P, RB], F32R, tag="qTs")
                        nc.sync.dma_start(
                            qTs[:],
                            qkv_dram[h, :, b * T + qb * RB : b * T + (qb + 1) * RB
                                     ].bitcast(F32R))
                        nkt = 4 * qb + 4
                        at = p2a.tile([P, KTB, RB], F32R, tag="at")
                        o_ps = p2ps_o.tile([P, RB], F32, tag="ops")
                        l_ps = p2ps_l.tile([P, RB], F32, tag="lps")
                        for kt in range(nkt):
                            s_ps = p2ps_s.tile([P, RB], F32, tag="sps")
                            nc.tensor.matmul(
                                s_ps[:], kT[:, kt * P : (kt + 1) * P], qTs[:],
                                start=True, stop=True)
                            if kt >= 4 * qb:
                                msk = p2t.tile([P, RB], F32, tag="msk")
                                nc.vector.tensor_tensor(
                                    msk[:], s_ps[:], mask_t[:, kt - 4 * qb], ADD)
                                esrc = msk
                            else:
                                esrc = s_ps
                            nc.scalar.activation(at[:, kt], esrc[:], EXP, scale=SCALE)
                            nc.tensor.matmul(
                                o_ps[:], v_rm[:, kt], at[:, kt],
                                start=(kt == 0), stop=(kt == nkt - 1))
                            nc.tensor.matmul(
                                l_ps[:], ones_t[:], at[:, kt],
                                start=(kt == 0), stop=(kt == nkt - 1))
                        rl = p2t.tile([P, RB], F32, tag="rl")
                        nc.vector.reciprocal(rl[:], l_ps[:])
                        ot = p2t.tile([P, RB], F32, tag="ot")
                        nc.vector.tensor_tensor(ot[:], o_ps[:], rl[:], MULT)
                        j = b * QBB + qb
                        nc.sync.dma_start(a2a_in[h][j, :, :], ot[:])
                    kv = kv_next
                    if b == B - 1:
                        nc.gpsimd.collective_compute(
                            "AllToAll", mybir.AluOpType.bypass,
                            ins=[a2a_in[h][:, :, :]], outs=[a2a_out[h][:, :, :]],
                            replica_groups=[list(range(NCORES))])
                        if h == 0:
                            # prefetch proj weights while A2A#1/h=1 attention run
                            wpre = cpool.tile([P, 2, DT, P], F32R)
                            for dd in range(2):
                                nc.sync.dma_start(
                                    wpre[:, dd], wproj[:, dd, :, :].bitcast(F32R))

            # ============ Phase 4: proj + residual (dt-halves) ============
            with tc.tile_pool(name="p4o", bufs=1) as p4o:
                x2n = p4o.tile([P, DT, RB], F32R)
                with (
                    tc.tile_pool(name="p45", bufs=1) as p45,
                    tc.tile_pool(name="p4w", bufs=2) as p4w,
                    tc.tile_pool(name="p4psA", bufs=2, space="PSUM") as p4psA,
                    tc.tile_pool(name="p4psB", bufs=1, space="PSUM") as p4psB,
                    tc.tile_pool(name="p45ps_ss", bufs=1, space="PSUM") as p45ps_ss,
                ):
                    otf = p45.tile([P, DT, RB], F32R)
                    for j in range(NCORES):
                        for hh in range(HPC):
                            nc.sync.dma_start(
                                otf[:, 2 * j + hh],
                                a2a_out[hh][j, :, :].bitcast(F32R))
                    x2 = p45.tile([P, DT, RB], F32)
                    for do in range(DT):
                        if do < 2:
                            wp = wpre[:, do]
                        else:
                            wpt = p4w.tile([P, DT, P], F32R, tag="wp", name=f"wp{do}")
                            nc.sync.dma_start(wpt[:], wproj[:, do, :, :].bitcast(F32R))
                            wp = wpt[:]
                        psA = p4psA.tile([P, RB], F32, tag="ppsA")
                        for dt in range(8):
                            nc.tensor.matmul(psA[:], wp[:, dt], otf[:, dt],
                                             start=(dt == 0), stop=(dt == 7))
                        psB = p4psB.tile([P, RB], F32, tag="ppsB")
                        for dt in range(8, DT):
                            nc.tensor.matmul(psB[:], wp[:, dt], otf[:, dt],
                                             start=(dt == 8), stop=(dt == DT - 1))
                        xr = p4w.tile([P, RB], F32, tag="xr")
                        nc.sync.dma_start(xr[:], xres[do, :, :])
                        nc.vector.tensor_tensor(x2[:, do], psA[:], xr[:], ADD)
                        nc.vector.tensor_tensor(x2[:, do], x2[:, do], psB[:], ADD)
                        nc.sync.dma_start(x2_dram[do, :, :], x2[:, do])

                    # ---- norm2 ----
                    ss2 = p45ps_ss.tile([P, RB], F32, tag="ss2")
                    sacc2 = p4w.tile([P, RB], F32R, tag="sacc2")
                    first = None
                    for dt in range(DT):
                        sq = p4w.tile([P, RB], F32R, tag="sq2")
                        nc.scalar.activation(sq[:], x2[:, dt], SQUARE)
                        if dt == 0:
                            first = sq
                        elif dt == 1:
                            nc.vector.tensor_tensor(sacc2[:], first[:], sq[:], ADD)
                        else:
                            nc.vector.tensor_tensor(sacc2[:], sacc2[:], sq[:], ADD)
                    nc.tensor.matmul(ss2[:], ones_t[:], sacc2[:], start=True, stop=True)
                    rms2 = p4w.tile([P, RB], F32, tag="rms2")
                    nc.vector.tensor_scalar(rms2[:], ss2[:], 1.0 / D, EPS, MULT, ADD)
                    nc.scalar.sqrt(rms2[:], rms2[:])
                    nc.vector.reciprocal(rms2[:], rms2[:])
                    for dt in range(DT):
                        nc.vector.tensor_tensor(x2n[:, dt], x2[:, dt], rms2[:], MULT)

                # ============ Phase 6: SwiGLU MLP (fg quarters) ============
                quarters = [(0, 11), (11, 22), (22, 33), (33, FGT)]
                out_acc = p4o.tile([P, DT, RB], F32)
                with (
                    tc.tile_pool(name="p6g", bufs=1) as p6g,
                    tc.tile_pool(name="p6w", bufs=2) as p6w,
                    tc.tile_pool(name="p6t", bufs=3) as p6t,
                    tc.tile_pool(name="p6ps_g", bufs=2, space="PSUM") as p6ps_g,
                    tc.tile_pool(name="p6ps_v", bufs=2, space="PSUM") as p6ps_v,
                    tc.tile_pool(name="p6ps_o", bufs=2, space="PSUM") as p6ps_o,
                ):
                    for qi, (fg0, fg1) in enumerate(quarters):
                        nq = fg1 - fg0
                        gt = p6g.tile([P, 11, RB], F32R, tag="gt")
                        for fi in range(nq):
                            fg = fg0 + fi
                            wg = p6w.tile([P, DT, P], F32R, tag="wg")
                            nc.sync.dma_start(wg[:], wgate[:, fg, :, :].bitcast(F32R))
                            wv = p6w.tile([P, DT, P], F32R, tag="wv")
                            nc.sync.dma_start(wv[:], wval[:, fg, :, :].bitcast(F32R))
                            g_ps = p6ps_g.tile([P, RB], F32, tag="gps")
                            for dt in range(DT):
                                nc.tensor.matmul(g_ps[:], wg[:, dt], x2n[:, dt],
                                                 start=(dt == 0), stop=(dt == DT - 1))
                            v_ps = p6ps_v.tile([P, RB], F32, tag="vps")
                            for dt in range(DT):
                                nc.tensor.matmul(v_ps[:], wv[:, dt], x2n[:, dt],
                                                 start=(dt == 0), stop=(dt == DT - 1))
                            sg = p6t.tile([P, RB], F32, tag="sg")
                            if SIM_SILU:
                                nc.scalar.activation(
                                    sg[:], g_ps[:],
                                    mybir.ActivationFunctionType.Sigmoid)
                                nc.vector.tensor_tensor(sg[:], sg[:], g_ps[:], MULT)
                            else:
                                nc.scalar.activation(sg[:], g_ps[:], SILU)
                            nc.vector.tensor_tensor(gt[:, fi], sg[:], v_ps[:], MULT)
                        for do in range(DT):
                            wm = p6w.tile([P, 11, P], F32R, tag="wm")
                            nc.sync.dma_start(
                                wm[:, :nq], wmlp[:, do, fg0:fg1, :].bitcast(F32R))
                            o_ps = p6ps_o.tile([P, RB], F32, tag="ops6")
                            for fi in range(nq):
                                nc.tensor.matmul(o_ps[:], wm[:, fi], gt[:, fi],
                                                 start=(fi == 0), stop=(fi == nq - 1))
                            if qi == 0:
                                xrr = p6t.tile([P, RB], F32, tag="xrr")
                                nc.sync.dma_start(xrr[:], x2_dram[do, :, :])
                                nc.vector.tensor_tensor(
                                    out_acc[:, do], o_ps[:], xrr[:], ADD)
                            elif qi < len(quarters) - 1:
                                nc.vector.tensor_tensor(
                                    out_acc[:, do], o_ps[:], out_acc[:, do], ADD)
                            else:
                                fin = p6t.tile([P, RB], F32, tag="fin")
                                nc.vector.tensor_tensor(
                                    fin[:], o_ps[:], out_acc[:, do], ADD)
                                nc.sync.dma_start(outT[do, :, :], fin[:])

    nc.compile()
    return nc


def _rope_tables():
    inv_freq = 1.0 / (ROPE_BASE ** (np.arange(0, DH, 2, dtype=np.float32) / DH))
    t = np.arange(T, dtype=np.float32)
    freqs = np.outer(t, inv_freq)
    emb = np.repeat(freqs, 2, axis=-1)  # [T, DH]
    return np.cos(emb).astype(np.float32), np.sin(emb).astype(np.float32)


def _tile4(w, n_out_tiles, n_in_tiles):
    """[F_out, D_in] -> [P(p of d-tile), F_out/P tiles, D_in/P tiles, P(c of f-tile)].

    Element [p, f, dt, c] = w[f*P + c, dt*P + p].
    """
    Fo, Di = w.shape
    assert Fo == n_out_tiles * P and Di == n_in_tiles * P
    # -> [f, c, dt, p] then transpose to [p, f, dt, c]
    v = w.reshape(n_out_tiles, P, n_in_tiles, P)
    return np.ascontiguousarray(v.transpose(3, 0, 2, 1))


def _prepare_inputs(x, norm1_w, norm2_w, c_attn_w, c_proj_w, c_gate_w, c_val_w,
                    c_mlp_proj_w):
    xf = np.ascontiguousarray(x.reshape(R, D).T)  # [D, R]
    cos, sin = _rope_tables()
    cosT = np.ascontiguousarray(cos.T)  # [DH, T]
    sinT = np.ascontiguousarray(sin.T)

    # rot-half signed permutation: (P @ q)[d] = -q[d+1] (d even), q[d-1] (d odd)
    rotP = np.zeros((P, P), np.float32)
    for d in range(0, P, 2):
        rotP[d, d + 1] = -1.0
        rotP[d + 1, d] = 1.0
    rotPT = np.ascontiguousarray(rotP.T)

    # additive causal masks for diagonal k-tiles, ST layout [k partition, q col]
    masks = np.zeros((P, QBB, RB), np.float32)
    for di in range(QBB):
        p_idx = np.arange(P)[:, None] + di * P
        c_idx = np.arange(RB)[None, :]
        masks[:, di, :] = np.where(p_idx <= c_idx, 0.0, NEG)

    ones_in = np.ones((P, P), np.float32)
    ident_in = np.eye(P, dtype=np.float32)

    w1 = norm1_w.astype(np.float32)
    w2 = norm2_w.astype(np.float32)
    attn_w = c_attn_w.astype(np.float32) * w1[None, :]     # fold norm1
    gate_w = c_gate_w.astype(np.float32) * w2[None, :]     # fold norm2
    val_w = c_val_w.astype(np.float32) * w2[None, :]

    gate_p = np.zeros((FGP, D), np.float32)
    gate_p[:FG] = gate_w
    val_p = np.zeros((FGP, D), np.float32)
    val_p[:FG] = val_w
    mlp_p = np.zeros((D, FGP), np.float32)
    mlp_p[:, :FG] = c_mlp_proj_w.astype(np.float32)

    wproj_t = _tile4(c_proj_w.astype(np.float32), DT, DT)
    wgate_t = _tile4(gate_p, FGT, DT)
    wval_t = _tile4(val_p, FGT, DT)
    # wmlp: lhsT [fg partition, dout col]: [p, do, fg, c] = mlp_p[do*P+c, fg*P+p]
    wmlp_t = np.ascontiguousarray(
        mlp_p.reshape(DT, P, FGT, P).transpose(3, 0, 2, 1)
    )

    in_maps = []
    for i in range(NCORES):
        h0, h1 = 2 * i, 2 * i + 1
        rows = []
        for base in (0, D, 2 * D):  # q, k, v row groups of c_attn_w
            rows.extend(range(base + h0 * DH, base + h0 * DH + DH))
            rows.extend(range(base + h1 * DH, base + h1 * DH + DH))
        wsel = attn_w[rows, :]                       # [768, D]
        wqkv_t = _tile4(wsel, NF, DT)
        xres_i = np.ascontiguousarray(
            xf[:, i * RB : (i + 1) * RB].reshape(DT, P, RB)
        )
        in_maps.append({
            "xT": xf,
            "xres": xres_i,
            "wqkv": wqkv_t,
            "wproj": wproj_t,
            "wgate": wgate_t,
            "wval": wval_t,
            "wmlp": wmlp_t,
            "cosT": cosT,
            "sinT": sinT,
            "rotPT": rotPT,
            "masks": masks,
            "ones_in": ones_in,
            "ident_in": ident_in,
        })
    return in_maps


_NC_CACHE = None


def _get_program():
    global _NC_CACHE
    if _NC_CACHE is None:
        _NC_CACHE = _build_program()
    return _NC_CACHE


def run(inputs, trace=False):
    """Returns (output [B,T,D], exec_time_ns or None)."""
    in_maps = _prepare_inputs(**inputs)
    nc = _get_program()
    res = run_bass_kernel_spmd(nc, in_maps, list(range(NCORES)), trace=trace)
    cols = []
    for i in range(NCORES):
        o = res.results[i]["outT"]          # [DT, P, RB]
        cols.append(o.reshape(D, RB))
    full_T = np.concatenate(cols, axis=1)   # [D, R]
    out = np.ascontiguousarray(full_T.T).reshape(B, T, D).astype(np.float32)
    return out, res.exec_time_ns


def kernel(**inputs) -> np.ndarray:
    out, _ = run(inputs, trace=False)
    return out



# revision 17
# speedup vs baseline: 1.1450x; 1.1450x over previous
"""Trainium2 Bass kernel for a dense pre-norm transformer block (B=2, T=2048,
D=2048, H=16, DH=128, FG=5461, SwiGLU MLP, RoPE, causal attention).

Sharding: tensor-parallel attention over heads (2 heads/core on 8 cores), one
AllToAll per head to reshard to 512 query columns per core, then fully local
proj + MLP per core (weights replicated, streamed from HBM).

All on-device activations are kept transposed ([feature, row]) so every
matmul is lhsT(=weight tile).T @ rhs(=activationT tile) with the contraction
dim on SBUF partitions.

v3: all big matmuls in bf16 (incl. QK^T); phase-1 qkv matmuls use 1024-wide
moving operands; A2A[h1] hidden under the h0 half of proj; approx
reciprocals; norm2 rms factored out of the MLP; rms partial-sum adds on
GpSimd (phase 1) / DVE (phase 4); residual kept in SBUF.
"""

import numpy as np
import ml_dtypes

import concourse.bass as bass
import concourse.mybir as mybir
import concourse.tile as tile
from concourse import bacc
from concourse.bass_utils import run_bass_kernel_spmd

# Problem constants
B, T, D = 2, 2048, 2048
H, DH = 16, 128
FG = 5461
EPS = 1e-5
ROPE_BASE = 10000.0

P = 128
NCORES = 8
R = B * T                    # 4096 rows total
RB = 512                     # rows per core / per-core q-cols
RB1 = 1024                   # phase-1 row block (bf16 moving max)
NRB1 = R // RB1              # 4 phase-1 row blocks
DT = D // P                  # 16 d-tiles
HPC = H // NCORES            # 2 heads per core
NF = 3 * HPC                 # 6 feature tiles per core in qkv (q0,q1,k0,k1,v0,v1)
FGP = 5504                   # FG padded to 43*128
FGT = FGP // P               # 43 fg tiles
KTB = T // P                 # 16 k-tiles per batch
QBB = T // RB                # 4 q-blocks per batch
NEG = -1.0e30
SCALE = 1.0 / np.sqrt(DH)

SIM_SILU = False  # replace Silu with sigmoid+mults (for CoreSim, which lacks Silu)
GP_ADDS = True    # run phase-1 rms partial adds on GpSimd (else DVE)

F32 = mybir.dt.float32
F32R = mybir.dt.float32r
BF16 = mybir.dt.bfloat16
EXP = mybir.ActivationFunctionType.Exp
SQUARE = mybir.ActivationFunctionType.Square
SILU = mybir.ActivationFunctionType.Silu
MULT = mybir.AluOpType.mult
ADD = mybir.AluOpType.add

NPBF16 = ml_dtypes.bfloat16


def _build_program():
    nc = bacc.Bacc("TRN2", target_bir_lowering=False, debug=False, num_devices=NCORES)

    # ---- per-core external inputs ----
    xT = nc.dram_tensor("xT", [D, R], BF16, kind="ExternalInput")
    xres = nc.dram_tensor("xres", [DT, P, RB], F32, kind="ExternalInput")
    wqkv = nc.dram_tensor("wqkv", [P, NF, DT, P], BF16, kind="ExternalInput")
    # wproj reordered host-side: [p, do, hh, j, c] with dt = 2*j + hh
    wproj = nc.dram_tensor("wproj", [P, DT, HPC, NCORES, P], BF16,
                           kind="ExternalInput")
    wgate = nc.dram_tensor("wgate", [P, FGT, DT, P], BF16, kind="ExternalInput")
    wval = nc.dram_tensor("wval", [P, FGT, DT, P], BF16, kind="ExternalInput")
    wmlp = nc.dram_tensor("wmlp", [P, DT, FGT, P], BF16, kind="ExternalInput")
    cosT = nc.dram_tensor("cosT", [P, T], F32, kind="ExternalInput")
    sinT = nc.dram_tensor("sinT", [P, T], F32, kind="ExternalInput")
    rotPT = nc.dram_tensor("rotPT", [P, P], F32, kind="ExternalInput")
    masks = nc.dram_tensor("masks", [P, QBB, RB], F32, kind="ExternalInput")
    ones_in = nc.dram_tensor("ones_in", [P, P], F32, kind="ExternalInput")
    ones_bf = nc.dram_tensor("ones_bf", [P, P], BF16, kind="ExternalInput")
    ident_bf = nc.dram_tensor("ident_bf", [P, P], BF16, kind="ExternalInput")

    outT = nc.dram_tensor("outT", [DT, P, RB], F32, kind="ExternalOutput")

    # ---- internal DRAM scratch ----
    qk_dram = nc.dram_tensor("qk_dram", [2 * HPC, P, R], BF16)  # q0,q1,k0,k1
    v_dram = nc.dram_tensor("v_dram", [HPC, P, R], BF16)        # v0,v1
    a2a_in = [nc.dram_tensor(f"a2a_in{h}", [NCORES, P, RB], BF16) for h in range(HPC)]
    a2a_out = [nc.dram_tensor(f"a2a_out{h}", [NCORES, P, RB], BF16) for h in range(HPC)]

    gp_add = nc.gpsimd.tensor_tensor if GP_ADDS else nc.vector.tensor_tensor

    with tile.TileContext(nc) as tc:
        with (
            tc.tile_pool(name="const", bufs=1) as cpool,
            tc.tile_pool(name="pkv0", bufs=1) as pkv0,
        ):
            rot_t = cpool.tile([P, P], F32R)
            ones_t = cpool.tile([P, P], F32R)
            onesb_t = cpool.tile([P, P], BF16)
            id_t = cpool.tile([P, P], BF16)
            nc.sync.dma_start(rot_t[:], rotPT[:, :].bitcast(F32R))
            nc.sync.dma_start(ones_t[:], ones_in[:, :].bitcast(F32R))
            nc.sync.dma_start(onesb_t[:], ones_bf[:, :])
            nc.sync.dma_start(id_t[:], ident_bf[:, :])

            # ============ Phase 1: qkv on raw x + rope, rms applied at output ==
            with (
                tc.tile_pool(name="p1c", bufs=1) as p1c,
                tc.tile_pool(name="p1w", bufs=1) as p1w,
                tc.tile_pool(name="p1x", bufs=3) as p1x,
                tc.tile_pool(name="p1t", bufs=2) as p1t,
                tc.tile_pool(name="p1sq", bufs=4) as p1sq,
                tc.tile_pool(name="p1sa", bufs=10) as p1sa,
                tc.tile_pool(name="p1ps_ss", bufs=2, space="PSUM") as p1ps_ss,
                tc.tile_pool(name="p1ps_mm", bufs=3, space="PSUM") as p1ps_mm,
                tc.tile_pool(name="p1ps_rot", bufs=2, space="PSUM") as p1ps_rot,
            ):
                wq_t = p1w.tile([P, NF, DT, P], BF16)
                xTr = xT.rearrange("(dt p) r -> p dt r", p=P)
                cos_t = p1c.tile([P, T], F32)
                sin_t = p1c.tile([P, T], F32)
                NRB = R // RB

                kb0 = pkv0.tile([P, T], BF16)
                vb0 = pkv0.tile([P, T], BF16)
                for rb in range(NRB):
                    t0 = (rb % QBB) * RB
                    xb = p1x.tile([P, DT, RB], BF16, tag="xblk")
                    nc.sync.dma_start(xb[:], xTr[:, :, rb * RB : (rb + 1) * RB])
                    if rb == 0:
                        # after xb0 on the FIFO queue: weights per-f (so f=0
                        # matmuls start early), then rope tables
                        for f in range(NF):
                            nc.sync.dma_start(wq_t[:, f], wqkv[:, f, :, :])
                        nc.sync.dma_start(cos_t[:], cosT[:, :])
                        nc.sync.dma_start(sin_t[:], sinT[:, :])
                    # rms: squares on ACT, partial-chain adds (4 sq each) on
                    # GpSimd, partials reduced via PSUM-accumulated ones-matmuls
                    ss_ps = p1ps_ss.tile([P, RB], F32, tag="ss")
                    for pp in range(4):
                        sp = p1sa.tile([P, RB], F32R, tag="sacc")
                        for k in range(4):
                            dt = pp * 4 + k
                            sq = p1sq.tile([P, RB], F32R, tag="sq")
                            nc.scalar.activation(sq[:], xb[:, dt], SQUARE)
                            if k == 0:
                                first = sq
                            elif k == 1:
                                gp_add(sp[:], first[:], sq[:], ADD)
                            else:
                                gp_add(sp[:], sp[:], sq[:], ADD)
                        nc.tensor.matmul(ss_ps[:], ones_t[:], sp[:],
                                         start=(pp == 0), stop=(pp == 3))
                    rms = p1t.tile([P, RB], F32, tag="rms")
                    nc.vector.tensor_scalar(rms[:], ss_ps[:], 1.0 / D, EPS, MULT, ADD)
                    nc.scalar.sqrt(rms[:], rms[:])
                    rmsr = p1t.tile([P, RB], F32, tag="rmsr")
                    nc.vector.reciprocal_approx_fast(rmsr[:], rms[:])
                    # qkv matmuls on RAW x; f: 0,1=q; 2,3=k; 4,5=v
                    for f in range(NF):
                        ps = p1ps_mm.tile([P, RB], F32, tag="qkvps")
                        for dt in range(DT):
                            nc.tensor.matmul(
                                ps[:], wq_t[:, f, dt], xb[:, dt],
                                start=(dt == 0), stop=(dt == DT - 1),
                            )
                        if f < 2 * HPC:
                            raw = p1t.tile([P, RB], F32R, tag="raw")
                            nc.vector.tensor_copy(raw[:], ps[:])
                            rps = p1ps_rot.tile([P, RB], F32, tag="rotps")
                            nc.tensor.matmul(rps[:], rot_t[:], raw[:],
                                             start=True, stop=True)
                            m1 = p1t.tile([P, RB], F32, tag="m1")
                            nc.vector.tensor_tensor(
                                m1[:], ps[:], cos_t[:, t0 : t0 + RB], MULT)
                            m2 = p1t.tile([P, RB], F32, tag="m2")
                            nc.vector.tensor_tensor(
                                m2[:], rps[:], sin_t[:, t0 : t0 + RB], MULT)
                            rr = p1t.tile([P, RB], F32, tag="rr")
                            nc.vector.tensor_tensor(rr[:], m1[:], m2[:], ADD)
                            if f == 2 and rb < QBB:
                                dst = kb0[:, rb * RB : (rb + 1) * RB]
                                nc.vector.tensor_tensor(dst, rr[:], rmsr[:], MULT)
                                nc.sync.dma_start(
                                    qk_dram[f, :, rb * RB : (rb + 1) * RB], dst)
                            else:
                                out_t = p1t.tile([P, RB], BF16, tag="outt")
                                nc.vector.tensor_tensor(out_t[:], rr[:], rmsr[:],
                                                        MULT)
                                nc.sync.dma_start(
                                    qk_dram[f, :, rb * RB : (rb + 1) * RB],
                                    out_t[:])
                        else:
                            if f == 4 and rb < QBB:
                                dst = vb0[:, rb * RB : (rb + 1) * RB]
                                nc.vector.tensor_tensor(dst, ps[:], rmsr[:], MULT)
                                nc.sync.dma_start(
                                    v_dram[0, :, rb * RB : (rb + 1) * RB], dst)
                            else:
                                outv = p1t.tile([P, RB], BF16, tag="outv")
                                nc.vector.tensor_tensor(outv[:], ps[:], rmsr[:],
                                                        MULT)
                                nc.sync.dma_start(
                                    v_dram[f - 2 * HPC, :,
                                           rb * RB : (rb + 1) * RB], outv[:])

            # ============ Phase 2: attention, h outer (A2A per head) ============
            # The h0 half of proj lives inside the attention scope so its
            # matmuls fill the A2A[h1] window. PSUM: tp1+s2+o2+l1+psA2 = 8.
            with tc.tile_pool(name="p46", bufs=1) as p46:
              x2b = p46.tile([P, DT, RB], BF16)
              rms2r = p46.tile([P, RB], F32)
              with tc.tile_pool(name="px2a", bufs=1) as px2a:
                x2a = px2a.tile([P, DT, RB], F32)
                with (
                    tc.tile_pool(name="p2c", bufs=1) as p2c,
                    tc.tile_pool(name="p2kv", bufs=2) as p2kv,
                    tc.tile_pool(name="p2a", bufs=3) as p2a,
                    tc.tile_pool(name="p2t", bufs=3) as p2t,
                    tc.tile_pool(name="p4a0", bufs=1) as p4a0,
                    tc.tile_pool(name="p4w0", bufs=3) as p4w0,
                    tc.tile_pool(name="p2ps_tp", bufs=2, space="PSUM") as p2ps_tp,
                    tc.tile_pool(name="p2ps_s", bufs=2, space="PSUM") as p2ps_s,
                    tc.tile_pool(name="p2ps_o", bufs=2, space="PSUM") as p2ps_o,
                    tc.tile_pool(name="p2ps_l", bufs=1, space="PSUM") as p2ps_l,
                    tc.tile_pool(name="p4psA", bufs=1, space="PSUM") as p4psA,
                ):
                    pairs = [(h, b) for h in range(HPC) for b in range(B)]

                    def load_kv(h, b):
                        kT = p2kv.tile([P, T], BF16, tag="kT", name=f"kT{h}{b}")
                        vT = p2kv.tile([P, T], BF16, tag="vT", name=f"vT{h}{b}")
                        nc.sync.dma_start(
                            kT[:], qk_dram[HPC + h, :, b * T : (b + 1) * T])
                        nc.sync.dma_start(vT[:], v_dram[h, :, b * T : (b + 1) * T])
                        return kT, vT

                    def transpose_v(vT, h, b):
                        v_rm = p2kv.tile([P, KTB, P], BF16, tag="v_rm",
                                         name=f"vrm{h}{b}")
                        for kt in range(KTB):
                            tps = p2ps_tp.tile([P, P], BF16, tag="vtp")
                            nc.tensor.transpose(
                                tps[:], vT[:, kt * P : (kt + 1) * P], id_t[:])
                            nc.vector.tensor_copy(v_rm[:, kt], tps[:])
                        return v_rm

                    kv = (kb0, vb0)
                    mask_t = p2c.tile([P, QBB, RB], F32)
                    nc.sync.dma_start(mask_t[:], masks[:, :, :])
                    for pi, (h, b) in enumerate(pairs):
                        kT, vT = kv
                        v_rm = transpose_v(vT, h, b)
                        kv_next = (load_kv(*pairs[pi + 1])
                                   if pi + 1 < len(pairs) else None)
                        for qb in range(QBB):
                            qTs = p2t.tile([P, RB], BF16, tag="qTs")
                            nc.sync.dma_start(
                                qTs[:],
                                qk_dram[h, :, b * T + qb * RB :
                                        b * T + (qb + 1) * RB])
                            nkt = 4 * qb + 4
                            at = p2a.tile([P, KTB, RB], BF16, tag="at")
                            o_ps = p2ps_o.tile([P, RB], F32, tag="ops")
                            l_ps = p2ps_l.tile([P, RB], F32, tag="lps")
                            for kt in range(nkt):
                                s_ps = p2ps_s.tile([P, RB], F32, tag="sps")
                                nc.tensor.matmul(
                                    s_ps[:], kT[:, kt * P : (kt + 1) * P], qTs[:],
                                    start=True, stop=True)
                                if kt >= 4 * qb:
                                    msk = p2t.tile([P, RB], F32, tag="msk")
                                    nc.vector.tensor_tensor(
                                        msk[:], s_ps[:], mask_t[:, kt - 4 * qb],
                                        ADD)
                                    esrc = msk
                                else:
                                    esrc = s_ps
                                nc.scalar.activation(at[:, kt], esrc[:], EXP,
                                                     scale=SCALE)
                                nc.tensor.matmul(
                                    o_ps[:], v_rm[:, kt], at[:, kt],
                                    start=(kt == 0), stop=(kt == nkt - 1))
                                nc.tensor.matmul(
                                    l_ps[:], onesb_t[:], at[:, kt],
                                    start=(kt == 0), stop=(kt == nkt - 1))
                            rl = p2t.tile([P, RB], F32, tag="rl")
                            nc.vector.reciprocal_approx_fast(rl[:], l_ps[:])
                            ot = p2t.tile([P, RB], BF16, tag="ot")
                            nc.vector.tensor_tensor(ot[:], o_ps[:], rl[:], MULT)
                            j = b * QBB + qb
                            nc.sync.dma_start(a2a_in[h][j, :, :], ot[:])
                        kv = kv_next
                        if b == B - 1:
                            nc.gpsimd.collective_compute(
                                "AllToAll", mybir.AluOpType.bypass,
                                ins=[a2a_in[h][:, :, :]],
                                outs=[a2a_out[h][:, :, :]],
                                replica_groups=[list(range(NCORES))])

                    # residual x slice (f32): DMA'd straight into x2a.
                    # On the ACT HWDGE queue: the sync queue is FIFO and a
                    # blocked A2A-gated load there head-blocks attention DMAs.
                    xresr = xres.rearrange("dt p rb -> p dt rb")
                    nc.scalar.dma_start(x2a[:], xresr[:, :, :])

                    # ---- proj h0 pass: consumes a2a_out[0]; overlaps late
                    # attention + A2A[h1]
                    otf0 = p4a0.tile([P, NCORES, RB], BF16)
                    for j in range(NCORES):
                        nc.scalar.dma_start(otf0[:, j], a2a_out[0][j, :, :])
                    for do in range(DT):
                        wpA = p4w0.tile([P, NCORES, P], BF16, tag="wp")
                        nc.sync.dma_start(wpA[:], wproj[:, do, 0, :, :])
                        psA = p4psA.tile([P, RB], F32, tag="ppsA")
                        for j in range(NCORES):
                            nc.tensor.matmul(psA[:], wpA[:, j], otf0[:, j],
                                             start=(j == 0),
                                             stop=(j == NCORES - 1))
                        nc.vector.tensor_tensor(x2a[:, do], psA[:],
                                                x2a[:, do], ADD)

                # ========= Phase 4b: proj h1 pass + norm2 ======================
                with (
                    tc.tile_pool(name="p4i", bufs=1) as p4i,
                    tc.tile_pool(name="p4w", bufs=3) as p4w,
                    tc.tile_pool(name="p4t", bufs=2) as p4t,
                    tc.tile_pool(name="p4sq", bufs=4) as p4sq,
                    tc.tile_pool(name="p4sa", bufs=6) as p4sa,
                    tc.tile_pool(name="p4psB", bufs=2, space="PSUM") as p4psB,
                    tc.tile_pool(name="p45ps_ss", bufs=1, space="PSUM") as p45ss,
                ):
                    otf1 = p4i.tile([P, NCORES, RB], BF16)
                    for j in range(NCORES):
                        nc.scalar.dma_start(otf1[:, j], a2a_out[1][j, :, :])
                    ss2 = p45ss.tile([P, RB], F32, tag="ss2")
                    sps = []
                    for do in range(DT):
                        wpB = p4w.tile([P, NCORES, P], BF16, tag="wp")
                        nc.sync.dma_start(wpB[:], wproj[:, do, 1, :, :])
                        psB = p4psB.tile([P, RB], F32, tag="ppsB")
                        for j in range(NCORES):
                            nc.tensor.matmul(psB[:], wpB[:, j], otf1[:, j],
                                             start=(j == 0),
                                             stop=(j == NCORES - 1))
                        nc.vector.tensor_tensor(x2b[:, do], x2a[:, do], psB[:],
                                                ADD)
                        sq = p4sq.tile([P, RB], F32R, tag="sq2")
                        nc.scalar.activation(sq[:], x2b[:, do], SQUARE)
                        k = do % 4
                        if k == 0:
                            first = sq
                        elif k == 1:
                            sp = p4sa.tile([P, RB], F32R, tag="sacc2")
                            gp_add(sp[:], first[:], sq[:], ADD)
                            sps.append(sp)
                        else:
                            gp_add(sps[-1][:], sps[-1][:], sq[:], ADD)
                        if k == 3:
                            pp = do // 4
                            nc.tensor.matmul(ss2[:], ones_t[:], sps[-1][:],
                                             start=(pp == 0), stop=(pp == 3))
                    rms2 = p4t.tile([P, RB], F32, tag="rms2")
                    nc.vector.tensor_scalar(rms2[:], ss2[:], 1.0 / D, EPS, MULT,
                                            ADD)
                    nc.scalar.sqrt(rms2[:], rms2[:])
                    nc.vector.reciprocal_approx_fast(rms2r[:], rms2[:])

              # ============ Phase 6: SwiGLU MLP (fg quarters) ============
              # gate/val matmuls consume un-normalized x2b; rms applied to the
              # silu input and once to the final output (row-scale commutes).
              quarters = [(0, 11), (11, 22), (22, 33), (33, FGT)]
              with (
                  tc.tile_pool(name="p6a", bufs=1) as p6a,
                  tc.tile_pool(name="p6g", bufs=1) as p6g,
                  tc.tile_pool(name="p6w", bufs=3) as p6w,
                  tc.tile_pool(name="p6t", bufs=3) as p6t,
                  tc.tile_pool(name="p6ps_g", bufs=2, space="PSUM") as p6ps_g,
                  tc.tile_pool(name="p6ps_v", bufs=2, space="PSUM") as p6ps_v,
                  tc.tile_pool(name="p6ps_o", bufs=1, space="PSUM") as p6ps_o,
              ):
                  out_acc = p6a.tile([P, DT, RB], F32)
                  for qi, (fg0, fg1) in enumerate(quarters):
                      nq = fg1 - fg0
                      gt = p6g.tile([P, 11, RB], BF16, tag="gt")
                      for fi in range(nq):
                          fg = fg0 + fi
                          wg = p6w.tile([P, DT, P], BF16, tag="wg")
                          nc.sync.dma_start(wg[:], wgate[:, fg, :, :])
                          wv = p6w.tile([P, DT, P], BF16, tag="wv")
                          nc.sync.dma_start(wv[:], wval[:, fg, :, :])
                          g_ps = p6ps_g.tile([P, RB], F32, tag="gps")
                          for dt in range(DT):
                              nc.tensor.matmul(g_ps[:], wg[:, dt], x2b[:, dt],
                                               start=(dt == 0),
                                               stop=(dt == DT - 1))
                          v_ps = p6ps_v.tile([P, RB], F32, tag="vps")
                          for dt in range(DT):
                              nc.tensor.matmul(v_ps[:], wv[:, dt], x2b[:, dt],
                                               start=(dt == 0),
                                               stop=(dt == DT - 1))
                          gn = p6t.tile([P, RB], F32, tag="gn")
                          nc.vector.tensor_tensor(gn[:], g_ps[:], rms2r[:], MULT)
                          sg = p6t.tile([P, RB], F32, tag="sg")
                          if SIM_SILU:
                              nc.scalar.activation(
                                  sg[:], gn[:],
                                  mybir.ActivationFunctionType.Sigmoid)
                              nc.vector.tensor_tensor(sg[:], sg[:], gn[:], MULT)
                          else:
                              nc.scalar.activation(sg[:], gn[:], SILU)
                          nc.vector.tensor_tensor(gt[:, fi], sg[:], v_ps[:], MULT)
                      for do in range(DT):
                          wm = p6w.tile([P, 11, P], BF16, tag="wm")
                          nc.sync.dma_start(wm[:, :nq], wmlp[:, do, fg0:fg1, :])
                          o_ps = p6ps_o.tile([P, RB], F32, tag="ops6")
                          for fi in range(nq):
                              nc.tensor.matmul(o_ps[:], wm[:, fi], gt[:, fi],
                                               start=(fi == 0),
                                               stop=(fi == nq - 1))
                          if qi == 0:
                              nc.vector.tensor_copy(out_acc[:, do], o_ps[:])
                          elif qi < len(quarters) - 1:
                              nc.vector.tensor_tensor(
                                  out_acc[:, do], o_ps[:], out_acc[:, do], ADD)
                          else:
                              tmp = p6t.tile([P, RB], F32, tag="tmp6")
                              nc.vector.tensor_tensor(
                                  tmp[:], o_ps[:], out_acc[:, do], ADD)
                              fin = p6t.tile([P, RB], F32, tag="fin")
                              nc.vector.tensor_tensor(fin[:], tmp[:], rms2r[:],
                                                      MULT)
                              nc.vector.tensor_tensor(fin[:], fin[:], x2b[:, do],
                                                      ADD)
                              nc.sync.dma_start(outT[do, :, :], fin[:])

    nc.compile()
    return nc


def _rope_tables():
    inv_freq = 1.0 / (ROPE_BASE ** (np.arange(0, DH, 2, dtype=np.float32) / DH))
    t = np.arange(T, dtype=np.float32)
    freqs = np.outer(t, inv_freq)
    emb = np.repeat(freqs, 2, axis=-1)  # [T, DH]
    return np.cos(emb).astype(np.float32), np.sin(emb).astype(np.float32)


def _tile4(w, n_out_tiles, n_in_tiles):
    """[F_out, D_in] -> [P(p of d-tile), F_out/P tiles, D_in/P tiles, P(c of f-tile)].

    Element [p, f, dt, c] = w[f*P + c, dt*P + p].
    """
    Fo, Di = w.shape
    assert Fo == n_out_tiles * P and Di == n_in_tiles * P
    v = w.reshape(n_out_tiles, P, n_in_tiles, P)
    return np.ascontiguousarray(v.transpose(3, 0, 2, 1))


def _prepare_inputs(x, norm1_w, norm2_w, c_attn_w, c_proj_w, c_gate_w, c_val_w,
                    c_mlp_proj_w):
    xf = np.ascontiguousarray(x.reshape(R, D).T)  # [D, R] f32
    xf_bf = xf.astype(NPBF16)
    cos, sin = _rope_tables()
    cosT = np.ascontiguousarray(cos.T)  # [DH, T]
    sinT = np.ascontiguousarray(sin.T)

    # rot-half signed permutation: (P @ q)[d] = -q[d+1] (d even), q[d-1] (d odd)
    rotP = np.zeros((P, P), np.float32)
    for d in range(0, P, 2):
        rotP[d, d + 1] = -1.0
        rotP[d + 1, d] = 1.0
    rotPT = np.ascontiguousarray(rotP.T)

    # additive causal masks for diagonal k-tiles, ST layout [k partition, q col]
    masks = np.zeros((P, QBB, RB), np.float32)
    for di in range(QBB):
        p_idx = np.arange(P)[:, None] + di * P
        c_idx = np.arange(RB)[None, :]
        masks[:, di, :] = np.where(p_idx <= c_idx, 0.0, NEG)

    ones_in = np.ones((P, P), np.float32)
    ones_bf = np.ones((P, P), NPBF16)
    ident_bf = np.eye(P).astype(NPBF16)

    w1 = norm1_w.astype(np.float32)
    w2 = norm2_w.astype(np.float32)
    attn_w = c_attn_w.astype(np.float32) * w1[None, :]     # fold norm1
    gate_w = c_gate_w.astype(np.float32) * w2[None, :]     # fold norm2
    val_w = c_val_w.astype(np.float32) * w2[None, :]

    gate_p = np.zeros((FGP, D), np.float32)
    gate_p[:FG] = gate_w
    val_p = np.zeros((FGP, D), np.float32)
    val_p[:FG] = val_w
    mlp_p = np.zeros((D, FGP), np.float32)
    mlp_p[:, :FG] = c_mlp_proj_w.astype(np.float32)

    # wproj: [p, do, dt, c] -> [p, do, hh, j, c] with dt = 2*j + hh
    wproj_t = _tile4(c_proj_w.astype(np.float32), DT, DT)
    wproj_r = np.ascontiguousarray(
        wproj_t.reshape(P, DT, NCORES, HPC, P).transpose(0, 1, 3, 2, 4)
    ).astype(NPBF16)
    wgate_t = _tile4(gate_p, FGT, DT).astype(NPBF16)
    wval_t = _tile4(val_p, FGT, DT).astype(NPBF16)
    # wmlp: lhsT [fg partition, dout col]: [p, do, fg, c] = mlp_p[do*P+c, fg*P+p]
    wmlp_t = np.ascontiguousarray(
        mlp_p.reshape(DT, P, FGT, P).transpose(3, 0, 2, 1)
    ).astype(NPBF16)

    in_maps = []
    for i in range(NCORES):
        h0, h1 = 2 * i, 2 * i + 1
        rows = []
        for base in (0, D, 2 * D):  # q, k, v row groups of c_attn_w
            rows.extend(range(base + h0 * DH, base + h0 * DH + DH))
            rows.extend(range(base + h1 * DH, base + h1 * DH + DH))
        wsel = attn_w[rows, :]                       # [768, D]
        wqkv_t = _tile4(wsel, NF, DT).astype(NPBF16)
        xres_i = np.ascontiguousarray(
            xf[:, i * RB : (i + 1) * RB].reshape(DT, P, RB)
        )
        in_maps.append({
            "xT": xf_bf,
            "xres": xres_i,
            "wqkv": wqkv_t,
            "wproj": wproj_r,
            "wgate": wgate_t,
            "wval": wval_t,
            "wmlp": wmlp_t,
            "cosT": cosT,
            "sinT": sinT,
            "rotPT": rotPT,
            "masks": masks,
            "ones_in": ones_in,
            "ones_bf": ones_bf,
            "ident_bf": ident_bf,
        })
    return in_maps


_NC_CACHE = None


def _get_program():
    global _NC_CACHE
    if _NC_CACHE is None:
        _NC_CACHE = _build_program()
    return _NC_CACHE


def run(inputs, trace=False):
    """Returns (output [B,T,D], exec_time_ns or None)."""
    in_maps = _prepare_inputs(**inputs)
    nc = _get_program()
    res = run_bass_kernel_spmd(nc, in_maps, list(range(NCORES)), trace=trace)
    cols = []
    for i in range(NCORES):
        o = res.results[i]["outT"]          # [DT, P, RB]
        cols.append(o.reshape(D, RB))
    full_T = np.concatenate(cols, axis=1)   # [D, R]
    out = np.ascontiguousarray(full_T.T).reshape(B, T, D).astype(np.float32)
    return out, res.exec_time_ns


def kernel(**inputs) -> np.ndarray:
    out, _ = run(inputs, trace=False)
    return out


# revision 18
# speedup vs baseline: 1.1491x; 1.0036x over previous
"""Trainium2 Bass kernel for a dense pre-norm transformer block (B=2, T=2048,
D=2048, H=16, DH=128, FG=5461, SwiGLU MLP, RoPE, causal attention).

Sharding: tensor-parallel attention over heads (2 heads/core on 8 cores), one
AllToAll per head to reshard to 512 query columns per core, then fully local
proj + MLP per core (weights replicated, streamed from HBM).

All on-device activations are kept transposed ([feature, row]) so every
matmul is lhsT(=weight tile).T @ rhs(=activationT tile) with the contraction
dim on SBUF partitions.

v3: all big matmuls in bf16 (incl. QK^T); phase-1 qkv matmuls use 1024-wide
moving operands; A2A[h1] hidden under the h0 half of proj; approx
reciprocals; norm2 rms factored out of the MLP; rms partial-sum adds on
GpSimd (phase 1) / DVE (phase 4); residual kept in SBUF.
"""

import numpy as np
import ml_dtypes

import concourse.bass as bass
import concourse.mybir as mybir
import concourse.tile as tile
from concourse import bacc
from concourse.bass_utils import run_bass_kernel_spmd

# Problem constants
B, T, D = 2, 2048, 2048
H, DH = 16, 128
FG = 5461
EPS = 1e-5
ROPE_BASE = 10000.0

P = 128
NCORES = 8
R = B * T                    # 4096 rows total
RB = 512                     # rows per core / per-core q-cols
RB1 = 1024                   # phase-1 row block (bf16 moving max)
NRB1 = R // RB1              # 4 phase-1 row blocks
DT = D // P                  # 16 d-tiles
HPC = H // NCORES            # 2 heads per core
NF = 3 * HPC                 # 6 feature tiles per core in qkv (q0,q1,k0,k1,v0,v1)
FGP = 5504                   # FG padded to 43*128
FGT = FGP // P               # 43 fg tiles
KTB = T // P                 # 16 k-tiles per batch
QBB = T // RB                # 4 q-blocks per batch
NEG = -1.0e30
SCALE = 1.0 / np.sqrt(DH)

SIM_SILU = False  # replace Silu with sigmoid+mults (for CoreSim, which lacks Silu)
GP_ADDS = True    # run phase-1 rms partial adds on GpSimd (else DVE)

F32 = mybir.dt.float32
F32R = mybir.dt.float32r
BF16 = mybir.dt.bfloat16
EXP = mybir.ActivationFunctionType.Exp
SQUARE = mybir.ActivationFunctionType.Square
SILU = mybir.ActivationFunctionType.Silu
MULT = mybir.AluOpType.mult
ADD = mybir.AluOpType.add

NPBF16 = ml_dtypes.bfloat16


def _build_program():
    nc = bacc.Bacc("TRN2", target_bir_lowering=False, debug=False, num_devices=NCORES)

    # ---- per-core external inputs ----
    xT = nc.dram_tensor("xT", [D, R], BF16, kind="ExternalInput")
    xres = nc.dram_tensor("xres", [DT, P, RB], F32, kind="ExternalInput")
    wqkv = nc.dram_tensor("wqkv", [P, NF, DT, P], BF16, kind="ExternalInput")
    # wproj reordered host-side: [p, do, hh, j, c] with dt = 2*j + hh
    wproj = nc.dram_tensor("wproj", [P, DT, HPC, NCORES, P], BF16,
                           kind="ExternalInput")
    wgate = nc.dram_tensor("wgate", [P, FGT, DT, P], BF16, kind="ExternalInput")
    wval = nc.dram_tensor("wval", [P, FGT, DT, P], BF16, kind="ExternalInput")
    wmlp = nc.dram_tensor("wmlp", [P, DT, FGT, P], BF16, kind="ExternalInput")
    cosT = nc.dram_tensor("cosT", [P, T], F32, kind="ExternalInput")
    sinT = nc.dram_tensor("sinT", [P, T], F32, kind="ExternalInput")
    rotPT = nc.dram_tensor("rotPT", [P, P], F32, kind="ExternalInput")
    masks = nc.dram_tensor("masks", [P, QBB, RB], F32, kind="ExternalInput")
    ones_in = nc.dram_tensor("ones_in", [P, P], F32, kind="ExternalInput")
    ones_bf = nc.dram_tensor("ones_bf", [P, P], BF16, kind="ExternalInput")
    ident_bf = nc.dram_tensor("ident_bf", [P, P], BF16, kind="ExternalInput")

    outT = nc.dram_tensor("outT", [DT, P, RB], F32, kind="ExternalOutput")

    # ---- internal DRAM scratch ----
    qk_dram = nc.dram_tensor("qk_dram", [2 * HPC, P, R], BF16)  # q0,q1,k0,k1
    v_dram = nc.dram_tensor("v_dram", [HPC, P, R], BF16)        # v0,v1
    a2a_in = [nc.dram_tensor(f"a2a_in{h}", [NCORES, P, RB], BF16) for h in range(HPC)]
    a2a_out = [nc.dram_tensor(f"a2a_out{h}", [NCORES, P, RB], BF16) for h in range(HPC)]

    gp_add = nc.gpsimd.tensor_tensor if GP_ADDS else nc.vector.tensor_tensor

    with tile.TileContext(nc) as tc:
        with (
            tc.tile_pool(name="const", bufs=1) as cpool,
            tc.tile_pool(name="pkv0", bufs=1) as pkv0,
        ):
            rot_t = cpool.tile([P, P], F32R)
            ones_t = cpool.tile([P, P], F32R)
            onesb_t = cpool.tile([P, P], BF16)
            id_t = cpool.tile([P, P], BF16)
            nc.sync.dma_start(rot_t[:], rotPT[:, :].bitcast(F32R))
            nc.sync.dma_start(ones_t[:], ones_in[:, :].bitcast(F32R))
            nc.sync.dma_start(onesb_t[:], ones_bf[:, :])
            nc.sync.dma_start(id_t[:], ident_bf[:, :])

            # ============ Phase 1: qkv on raw x + rope, rms applied at output ==
            with (
                tc.tile_pool(name="p1c", bufs=1) as p1c,
                tc.tile_pool(name="p1w", bufs=1) as p1w,
                tc.tile_pool(name="p1x", bufs=3) as p1x,
                tc.tile_pool(name="p1t", bufs=2) as p1t,
                tc.tile_pool(name="p1sq", bufs=4) as p1sq,
                tc.tile_pool(name="p1sa", bufs=10) as p1sa,
                tc.tile_pool(name="p1ps_ss", bufs=2, space="PSUM") as p1ps_ss,
                tc.tile_pool(name="p1ps_mm", bufs=3, space="PSUM") as p1ps_mm,
                tc.tile_pool(name="p1ps_rot", bufs=2, space="PSUM") as p1ps_rot,
            ):
                wq_t = p1w.tile([P, NF, DT, P], BF16)
                xTr = xT.rearrange("(dt p) r -> p dt r", p=P)
                cos_t = p1c.tile([P, T], F32)
                sin_t = p1c.tile([P, T], F32)
                NRB = R // RB

                kb0 = pkv0.tile([P, T], BF16)
                vb0 = pkv0.tile([P, T], BF16)
                for rb in range(NRB):
                    t0 = (rb % QBB) * RB
                    xb = p1x.tile([P, DT, RB], BF16, tag="xblk")
                    nc.sync.dma_start(xb[:], xTr[:, :, rb * RB : (rb + 1) * RB])
                    if rb == 0:
                        # after xb0 on the FIFO queue: weights per-f (so f=0
                        # matmuls start early), then rope tables
                        for f in range(NF):
                            nc.sync.dma_start(wq_t[:, f], wqkv[:, f, :, :])
                        nc.sync.dma_start(cos_t[:], cosT[:, :])
                        nc.sync.dma_start(sin_t[:], sinT[:, :])
                    # rms: squares on ACT, partial-chain adds (4 sq each) on
                    # GpSimd, partials reduced via PSUM-accumulated ones-matmuls
                    ss_ps = p1ps_ss.tile([P, RB], F32, tag="ss")
                    for pp in range(4):
                        sp = p1sa.tile([P, RB], F32R, tag="sacc")
                        for k in range(4):
                            dt = pp * 4 + k
                            sq = p1sq.tile([P, RB], F32R, tag="sq")
                            nc.scalar.activation(sq[:], xb[:, dt], SQUARE)
                            if k == 0:
                                first = sq
                            elif k == 1:
                                gp_add(sp[:], first[:], sq[:], ADD)
                            else:
                                gp_add(sp[:], sp[:], sq[:], ADD)
                        nc.tensor.matmul(ss_ps[:], ones_t[:], sp[:],
                                         start=(pp == 0), stop=(pp == 3))
                    rms = p1t.tile([P, RB], F32, tag="rms")
                    nc.vector.tensor_scalar(rms[:], ss_ps[:], 1.0 / D, EPS, MULT, ADD)
                    nc.scalar.sqrt(rms[:], rms[:])
                    rmsr = p1t.tile([P, RB], F32, tag="rmsr")
                    nc.vector.reciprocal_approx_fast(rmsr[:], rms[:])
                    # qkv matmuls on RAW x; f: 0,1=q; 2,3=k; 4,5=v
                    for f in range(NF):
                        ps = p1ps_mm.tile([P, RB], F32, tag="qkvps")
                        for dt in range(DT):
                            nc.tensor.matmul(
                                ps[:], wq_t[:, f, dt], xb[:, dt],
                                start=(dt == 0), stop=(dt == DT - 1),
                            )
                        if f < 2 * HPC:
                            raw = p1t.tile([P, RB], F32R, tag="raw")
                            nc.vector.tensor_copy(raw[:], ps[:])
                            rps = p1ps_rot.tile([P, RB], F32, tag="rotps")
                            nc.tensor.matmul(rps[:], rot_t[:], raw[:],
                                             start=True, stop=True)
                            m1 = p1t.tile([P, RB], F32, tag="m1")
                            nc.vector.tensor_tensor(
                                m1[:], ps[:], cos_t[:, t0 : t0 + RB], MULT)
                            m2 = p1t.tile([P, RB], F32, tag="m2")
                            nc.vector.tensor_tensor(
                                m2[:], rps[:], sin_t[:, t0 : t0 + RB], MULT)
                            rr = p1t.tile([P, RB], F32, tag="rr")
                            nc.vector.tensor_tensor(rr[:], m1[:], m2[:], ADD)
                            if f == 2 and rb < QBB:
                                dst = kb0[:, rb * RB : (rb + 1) * RB]
                                nc.vector.tensor_tensor(dst, rr[:], rmsr[:], MULT)
                                nc.sync.dma_start(
                                    qk_dram[f, :, rb * RB : (rb + 1) * RB], dst)
                            else:
                                out_t = p1t.tile([P, RB], BF16, tag="outt")
                                nc.vector.tensor_tensor(out_t[:], rr[:], rmsr[:],
                                                        MULT)
                                nc.sync.dma_start(
                                    qk_dram[f, :, rb * RB : (rb + 1) * RB],
                                    out_t[:])
                        else:
                            if f == 4 and rb < QBB:
                                dst = vb0[:, rb * RB : (rb + 1) * RB]
                                nc.vector.tensor_tensor(dst, ps[:], rmsr[:], MULT)
                                nc.sync.dma_start(
                                    v_dram[0, :, rb * RB : (rb + 1) * RB], dst)
                            else:
                                outv = p1t.tile([P, RB], BF16, tag="outv")
                                nc.vector.tensor_tensor(outv[:], ps[:], rmsr[:],
                                                        MULT)
                                nc.sync.dma_start(
                                    v_dram[f - 2 * HPC, :,
                                           rb * RB : (rb + 1) * RB], outv[:])

            # ============ Phase 2: attention, h outer (A2A per head) ============
            # The h0 half of proj lives inside the attention scope so its
            # matmuls fill the A2A[h1] window. PSUM: tp1+s2+o2+l1+psA2 = 8.
            with tc.tile_pool(name="p46", bufs=1) as p46:
              x2b = p46.tile([P, DT, RB], BF16)
              rms2r = p46.tile([P, RB], F32)
              with tc.tile_pool(name="px2a", bufs=1) as px2a:
                x2a = px2a.tile([P, DT, RB], F32)
                with (
                    tc.tile_pool(name="p2c", bufs=1) as p2c,
                    tc.tile_pool(name="p2kv", bufs=2) as p2kv,
                    tc.tile_pool(name="p2a", bufs=3) as p2a,
                    tc.tile_pool(name="p2t", bufs=3) as p2t,
                    tc.tile_pool(name="p4a0", bufs=1) as p4a0,
                    tc.tile_pool(name="p4w0", bufs=3) as p4w0,
                    tc.tile_pool(name="p2ps_tp", bufs=2, space="PSUM") as p2ps_tp,
                    tc.tile_pool(name="p2ps_s", bufs=2, space="PSUM") as p2ps_s,
                    tc.tile_pool(name="p2ps_o", bufs=2, space="PSUM") as p2ps_o,
                    tc.tile_pool(name="p2ps_l", bufs=1, space="PSUM") as p2ps_l,
                    tc.tile_pool(name="p4psA", bufs=1, space="PSUM") as p4psA,
                ):
                    pairs = [(h, b) for h in range(HPC) for b in range(B)]

                    def load_kv(h, b):
                        kT = p2kv.tile([P, T], BF16, tag="kT", name=f"kT{h}{b}")
                        vT = p2kv.tile([P, T], BF16, tag="vT", name=f"vT{h}{b}")
                        nc.sync.dma_start(
                            kT[:], qk_dram[HPC + h, :, b * T : (b + 1) * T])
                        nc.sync.dma_start(vT[:], v_dram[h, :, b * T : (b + 1) * T])
                        return kT, vT

                    def transpose_v(vT, h, b):
                        v_rm = p2kv.tile([P, KTB, P], BF16, tag="v_rm",
                                         name=f"vrm{h}{b}")
                        for kt in range(KTB):
                            tps = p2ps_tp.tile([P, P], BF16, tag="vtp")
                            nc.tensor.transpose(
                                tps[:], vT[:, kt * P : (kt + 1) * P], id_t[:])
                            nc.vector.tensor_copy(v_rm[:, kt], tps[:])
                        return v_rm

                    kv = (kb0, vb0)
                    mask_t = p2c.tile([P, QBB, RB], F32)
                    nc.sync.dma_start(mask_t[:], masks[:, :, :])
                    for pi, (h, b) in enumerate(pairs):
                        kT, vT = kv
                        v_rm = transpose_v(vT, h, b)
                        kv_next = (load_kv(*pairs[pi + 1])
                                   if pi + 1 < len(pairs) else None)
                        for qb in range(QBB):
                            qTs = p2t.tile([P, RB], BF16, tag="qTs")
                            nc.sync.dma_start(
                                qTs[:],
                                qk_dram[h, :, b * T + qb * RB :
                                        b * T + (qb + 1) * RB])
                            nkt = 4 * qb + 4
                            at = p2a.tile([P, KTB, RB], BF16, tag="at")
                            o_ps = p2ps_o.tile([P, RB], F32, tag="ops")
                            l_ps = p2ps_l.tile([P, RB], F32, tag="lps")
                            for kt in range(nkt):
                                s_ps = p2ps_s.tile([P, RB], F32, tag="sps")
                                nc.tensor.matmul(
                                    s_ps[:], kT[:, kt * P : (kt + 1) * P], qTs[:],
                                    start=True, stop=True)
                                if kt >= 4 * qb:
                                    msk = p2t.tile([P, RB], F32, tag="msk")
                                    nc.vector.tensor_tensor(
                                        msk[:], s_ps[:], mask_t[:, kt - 4 * qb],
                                        ADD)
                                    esrc = msk
                                else:
                                    esrc = s_ps
                                nc.scalar.activation(at[:, kt], esrc[:], EXP,
                                                     scale=SCALE)
                                nc.tensor.matmul(
                                    o_ps[:], v_rm[:, kt], at[:, kt],
                                    start=(kt == 0), stop=(kt == nkt - 1))
                                nc.tensor.matmul(
                                    l_ps[:], onesb_t[:], at[:, kt],
                                    start=(kt == 0), stop=(kt == nkt - 1))
                            rl = p2t.tile([P, RB], F32, tag="rl")
                            nc.vector.reciprocal_approx_fast(rl[:], l_ps[:])
                            ot = p2t.tile([P, RB], BF16, tag="ot")
                            nc.vector.tensor_tensor(ot[:], o_ps[:], rl[:], MULT)
                            j = b * QBB + qb
                            nc.sync.dma_start(a2a_in[h][j, :, :], ot[:])
                        kv = kv_next
                        if b == B - 1:
                            nc.gpsimd.collective_compute(
                                "AllToAll", mybir.AluOpType.bypass,
                                ins=[a2a_in[h][:, :, :]],
                                outs=[a2a_out[h][:, :, :]],
                                replica_groups=[list(range(NCORES))])

                    # residual x slice (f32): DMA'd straight into x2a.
                    # On the ACT HWDGE queue: the sync queue is FIFO and a
                    # blocked A2A-gated load there head-blocks attention DMAs.
                    xresr = xres.rearrange("dt p rb -> p dt rb")
                    nc.sync.dma_start(x2a[:], xresr[:, :, :])

                    # ---- proj h0 pass: consumes a2a_out[0]; overlaps late
                    # attention + A2A[h1]
                    otf0 = p4a0.tile([P, NCORES, RB], BF16)
                    for j in range(NCORES):
                        nc.gpsimd.dma_start(otf0[:, j], a2a_out[0][j, :, :])
                    for do in range(DT):
                        wpA = p4w0.tile([P, NCORES, P], BF16, tag="wp")
                        nc.sync.dma_start(wpA[:], wproj[:, do, 0, :, :])
                        psA = p4psA.tile([P, RB], F32, tag="ppsA")
                        for j in range(NCORES):
                            nc.tensor.matmul(psA[:], wpA[:, j], otf0[:, j],
                                             start=(j == 0),
                                             stop=(j == NCORES - 1))
                        nc.vector.tensor_tensor(x2a[:, do], psA[:],
                                                x2a[:, do], ADD)

                # ========= Phase 4b: proj h1 pass + norm2 ======================
                with (
                    tc.tile_pool(name="p4i", bufs=1) as p4i,
                    tc.tile_pool(name="p4w", bufs=3) as p4w,
                    tc.tile_pool(name="p4t", bufs=2) as p4t,
                    tc.tile_pool(name="p4sq", bufs=4) as p4sq,
                    tc.tile_pool(name="p4sa", bufs=6) as p4sa,
                    tc.tile_pool(name="p4psB", bufs=2, space="PSUM") as p4psB,
                    tc.tile_pool(name="p45ps_ss", bufs=1, space="PSUM") as p45ss,
                ):
                    otf1 = p4i.tile([P, NCORES, RB], BF16)
                    for j in range(NCORES):
                        nc.gpsimd.dma_start(otf1[:, j], a2a_out[1][j, :, :])
                    ss2 = p45ss.tile([P, RB], F32, tag="ss2")
                    sps = []
                    for do in range(DT):
                        wpB = p4w.tile([P, NCORES, P], BF16, tag="wp")
                        nc.sync.dma_start(wpB[:], wproj[:, do, 1, :, :])
                        psB = p4psB.tile([P, RB], F32, tag="ppsB")
                        for j in range(NCORES):
                            nc.tensor.matmul(psB[:], wpB[:, j], otf1[:, j],
                                             start=(j == 0),
                                             stop=(j == NCORES - 1))
                        nc.vector.tensor_tensor(x2b[:, do], x2a[:, do], psB[:],
                                                ADD)
                        sq = p4sq.tile([P, RB], F32R, tag="sq2")
                        nc.scalar.activation(sq[:], x2b[:, do], SQUARE)
                        k = do % 4
                        if k == 0:
                            first = sq
                        elif k == 1:
                            sp = p4sa.tile([P, RB], F32R, tag="sacc2")
                            gp_add(sp[:], first[:], sq[:], ADD)
                            sps.append(sp)
                        else:
                            gp_add(sps[-1][:], sps[-1][:], sq[:], ADD)
                        if k == 3:
                            pp = do // 4
                            nc.tensor.matmul(ss2[:], ones_t[:], sps[-1][:],
                                             start=(pp == 0), stop=(pp == 3))
                    rms2 = p4t.tile([P, RB], F32, tag="rms2")
                    nc.vector.tensor_scalar(rms2[:], ss2[:], 1.0 / D, EPS, MULT,
                                            ADD)
                    nc.scalar.sqrt(rms2[:], rms2[:])
                    nc.vector.reciprocal_approx_fast(rms2r[:], rms2[:])

              # ============ Phase 6: SwiGLU MLP (fg quarters) ============
              # gate/val matmuls consume un-normalized x2b; rms applied to the
              # silu input and once to the final output (row-scale commutes).
              quarters = [(0, 11), (11, 22), (22, 33), (33, FGT)]
              with (
                  tc.tile_pool(name="p6a", bufs=1) as p6a,
                  tc.tile_pool(name="p6g", bufs=1) as p6g,
                  tc.tile_pool(name="p6w", bufs=3) as p6w,
                  tc.tile_pool(name="p6t", bufs=3) as p6t,
                  tc.tile_pool(name="p6ps_g", bufs=2, space="PSUM") as p6ps_g,
                  tc.tile_pool(name="p6ps_v", bufs=2, space="PSUM") as p6ps_v,
                  tc.tile_pool(name="p6ps_o", bufs=1, space="PSUM") as p6ps_o,
              ):
                  out_acc = p6a.tile([P, DT, RB], F32)
                  for qi, (fg0, fg1) in enumerate(quarters):
                      nq = fg1 - fg0
                      gt = p6g.tile([P, 11, RB], BF16, tag="gt")
                      for fi in range(nq):
                          fg = fg0 + fi
                          wg = p6w.tile([P, DT, P], BF16, tag="wg")
                          nc.sync.dma_start(wg[:], wgate[:, fg, :, :])
                          wv = p6w.tile([P, DT, P], BF16, tag="wv")
                          nc.sync.dma_start(wv[:], wval[:, fg, :, :])
                          g_ps = p6ps_g.tile([P, RB], F32, tag="gps")
                          for dt in range(DT):
                              nc.tensor.matmul(g_ps[:], wg[:, dt], x2b[:, dt],
                                               start=(dt == 0),
                                               stop=(dt == DT - 1))
                          v_ps = p6ps_v.tile([P, RB], F32, tag="vps")
                          for dt in range(DT):
                              nc.tensor.matmul(v_ps[:], wv[:, dt], x2b[:, dt],
                                               start=(dt == 0),
                                               stop=(dt == DT - 1))
                          gn = p6t.tile([P, RB], F32, tag="gn")
                          nc.vector.tensor_tensor(gn[:], g_ps[:], rms2r[:], MULT)
                          sg = p6t.tile([P, RB], F32, tag="sg")
                          if SIM_SILU:
                              nc.scalar.activation(
                                  sg[:], gn[:],
                                  mybir.ActivationFunctionType.Sigmoid)
                              nc.vector.tensor_tensor(sg[:], sg[:], gn[:], MULT)
                          else:
                              nc.scalar.activation(sg[:], gn[:], SILU)
                          nc.vector.tensor_tensor(gt[:, fi], sg[:], v_ps[:], MULT)
                      for do in range(DT):
                          wm = p6w.tile([P, 11, P], BF16, tag="wm")
                          nc.sync.dma_start(wm[:, :nq], wmlp[:, do, fg0:fg1, :])
                          o_ps = p6ps_o.tile([P, RB], F32, tag="ops6")
                          for fi in range(nq):
                              nc.tensor.matmul(o_ps[:], wm[:, fi], gt[:, fi],
                                               start=(fi == 0),
                                               stop=(fi == nq - 1))
                          if qi == 0:
                              nc.vector.tensor_copy(out_acc[:, do], o_ps[:])
                          elif qi < len(quarters) - 1:
                              nc.vector.tensor_tensor(
                                  out_acc[:, do], o_ps[:], out_acc[:, do], ADD)
                          else:
                              tmp = p6t.tile([P, RB], F32, tag="tmp6")
                              nc.vector.tensor_tensor(
                                  tmp[:], o_ps[:], out_acc[:, do], ADD)
                              fin = p6t.tile([P, RB], F32, tag="fin")
                              nc.vector.tensor_tensor(fin[:], tmp[:], rms2r[:],
                                                      MULT)
                              nc.vector.tensor_tensor(fin[:], fin[:], x2b[:, do],
                                                      ADD)
                              nc.sync.dma_start(outT[do, :, :], fin[:])

    nc.compile()
    return nc


def _rope_tables():
    inv_freq = 1.0 / (ROPE_BASE ** (np.arange(0, DH, 2, dtype=np.float32) / DH))
    t = np.arange(T, dtype=np.float32)
    freqs = np.outer(t, inv_freq)
    emb = np.repeat(freqs, 2, axis=-1)  # [T, DH]
    return np.cos(emb).astype(np.float32), np.sin(emb).astype(np.float32)


def _tile4(w, n_out_tiles, n_in_tiles):
    """[F_out, D_in] -> [P(p of d-tile), F_out/P tiles, D_in/P tiles, P(c of f-tile)].

    Element [p, f, dt, c] = w[f*P + c, dt*P + p].
    """
    Fo, Di = w.shape
    assert Fo == n_out_tiles * P and Di == n_in_tiles * P
    v = w.reshape(n_out_tiles, P, n_in_tiles, P)
    return np.ascontiguousarray(v.transpose(3, 0, 2, 1))


def _prepare_inputs(x, norm1_w, norm2_w, c_attn_w, c_proj_w, c_gate_w, c_val_w,
                    c_mlp_proj_w):
    xf = np.ascontiguousarray(x.reshape(R, D).T)  # [D, R] f32
    xf_bf = xf.astype(NPBF16)
    cos, sin = _rope_tables()
    cosT = np.ascontiguousarray(cos.T)  # [DH, T]
    sinT = np.ascontiguousarray(sin.T)

    # rot-half signed permutation: (P @ q)[d] = -q[d+1] (d even), q[d-1] (d odd)
    rotP = np.zeros((P, P), np.float32)
    for d in range(0, P, 2):
        rotP[d, d + 1] = -1.0
        rotP[d + 1, d] = 1.0
    rotPT = np.ascontiguousarray(rotP.T)

    # additive causal masks for diagonal k-tiles, ST layout [k partition, q col]
    masks = np.zeros((P, QBB, RB), np.float32)
    for di in range(QBB):
        p_idx = np.arange(P)[:, None] + di * P
        c_idx = np.arange(RB)[None, :]
        masks[:, di, :] = np.where(p_idx <= c_idx, 0.0, NEG)

    ones_in = np.ones((P, P), np.float32)
    ones_bf = np.ones((P, P), NPBF16)
    ident_bf = np.eye(P).astype(NPBF16)

    w1 = norm1_w.astype(np.float32)
    w2 = norm2_w.astype(np.float32)
    attn_w = c_attn_w.astype(np.float32) * w1[None, :]     # fold norm1
    gate_w = c_gate_w.astype(np.float32) * w2[None, :]     # fold norm2
    val_w = c_val_w.astype(np.float32) * w2[None, :]

    gate_p = np.zeros((FGP, D), np.float32)
    gate_p[:FG] = gate_w
    val_p = np.zeros((FGP, D), np.float32)
    val_p[:FG] = val_w
    mlp_p = np.zeros((D, FGP), np.float32)
    mlp_p[:, :FG] = c_mlp_proj_w.astype(np.float32)

    # wproj: [p, do, dt, c] -> [p, do, hh, j, c] with dt = 2*j + hh
    wproj_t = _tile4(c_proj_w.astype(np.float32), DT, DT)
    wproj_r = np.ascontiguousarray(
        wproj_t.reshape(P, DT, NCORES, HPC, P).transpose(0, 1, 3, 2, 4)
    ).astype(NPBF16)
    wgate_t = _tile4(gate_p, FGT, DT).astype(NPBF16)
    wval_t = _tile4(val_p, FGT, DT).astype(NPBF16)
    # wmlp: lhsT [fg partition, dout col]: [p, do, fg, c] = mlp_p[do*P+c, fg*P+p]
    wmlp_t = np.ascontiguousarray(
        mlp_p.reshape(DT, P, FGT, P).transpose(3, 0, 2, 1)
    ).astype(NPBF16)

    in_maps = []
    for i in range(NCORES):
        h0, h1 = 2 * i, 2 * i + 1
        rows = []
        for base in (0, D, 2 * D):  # q, k, v row groups of c_attn_w
            rows.extend(range(base + h0 * DH, base + h0 * DH + DH))
            rows.extend(range(base + h1 * DH, base + h1 * DH + DH))
        wsel = attn_w[rows, :]                       # [768, D]
        wqkv_t = _tile4(wsel, NF, DT).astype(NPBF16)
        xres_i = np.ascontiguousarray(
            xf[:, i * RB : (i + 1) * RB].reshape(DT, P, RB)
        )
        in_maps.append({
            "xT": xf_bf,
            "xres": xres_i,
            "wqkv": wqkv_t,
            "wproj": wproj_r,
            "wgate": wgate_t,
            "wval": wval_t,
            "wmlp": wmlp_t,
            "cosT": cosT,
            "sinT": sinT,
            "rotPT": rotPT,
            "masks": masks,
            "ones_in": ones_in,
            "ones_bf": ones_bf,
            "ident_bf": ident_bf,
        })
    return in_maps


_NC_CACHE = None


def _get_program():
    global _NC_CACHE
    if _NC_CACHE is None:
        _NC_CACHE = _build_program()
    return _NC_CACHE


def run(inputs, trace=False):
    """Returns (output [B,T,D], exec_time_ns or None)."""
    in_maps = _prepare_inputs(**inputs)
    nc = _get_program()
    res = run_bass_kernel_spmd(nc, in_maps, list(range(NCORES)), trace=trace)
    cols = []
    for i in range(NCORES):
        o = res.results[i]["outT"]          # [DT, P, RB]
        cols.append(o.reshape(D, RB))
    full_T = np.concatenate(cols, axis=1)   # [D, R]
    out = np.ascontiguousarray(full_T.T).reshape(B, T, D).astype(np.float32)
    return out, res.exec_time_ns


def kernel(**inputs) -> np.ndarray:
    out, _ = run(inputs, trace=False)
    return out


# revision 20
# speedup vs baseline: 1.1539x; 1.0042x over previous
"""Trainium2 Bass kernel for a dense pre-norm transformer block (B=2, T=2048,
D=2048, H=16, DH=128, FG=5461, SwiGLU MLP, RoPE, causal attention).

Sharding: tensor-parallel attention over heads (2 heads/core on 8 cores), one
AllToAll per head to reshard to 512 query columns per core, then fully local
proj + MLP per core (weights replicated, streamed from HBM).

All on-device activations are kept transposed ([feature, row]) so every
matmul is lhsT(=weight tile).T @ rhs(=activationT tile) with the contraction
dim on SBUF partitions.

v3: all big matmuls in bf16 (incl. QK^T); phase-1 qkv matmuls use 1024-wide
moving operands; A2A[h1] hidden under the h0 half of proj; approx
reciprocals; norm2 rms factored out of the MLP; rms partial-sum adds on
GpSimd (phase 1) / DVE (phase 4); residual kept in SBUF.
"""

import numpy as np
import ml_dtypes

import concourse.bass as bass
import concourse.mybir as mybir
import concourse.tile as tile
from concourse.tile import add_dep_helper
from concourse import bacc
from concourse.bass_utils import run_bass_kernel_spmd

# Problem constants
B, T, D = 2, 2048, 2048
H, DH = 16, 128
FG = 5461
EPS = 1e-5
ROPE_BASE = 10000.0

P = 128
NCORES = 8
R = B * T                    # 4096 rows total
RB = 512                     # rows per core / per-core q-cols
RB1 = 1024                   # phase-1 row block (bf16 moving max)
NRB1 = R // RB1              # 4 phase-1 row blocks
DT = D // P                  # 16 d-tiles
HPC = H // NCORES            # 2 heads per core
NF = 3 * HPC                 # 6 feature tiles per core in qkv (q0,q1,k0,k1,v0,v1)
FGP = 5504                   # FG padded to 43*128
FGT = FGP // P               # 43 fg tiles
KTB = T // P                 # 16 k-tiles per batch
QBB = T // RB                # 4 q-blocks per batch
NEG = -1.0e30
SCALE = 1.0 / np.sqrt(DH)

SIM_SILU = False  # replace Silu with sigmoid+mults (for CoreSim, which lacks Silu)
GP_ADDS = True    # run phase-1 rms partial adds on GpSimd (else DVE)

F32 = mybir.dt.float32
F32R = mybir.dt.float32r
BF16 = mybir.dt.bfloat16
EXP = mybir.ActivationFunctionType.Exp
SQUARE = mybir.ActivationFunctionType.Square
SILU = mybir.ActivationFunctionType.Silu
MULT = mybir.AluOpType.mult
ADD = mybir.AluOpType.add

NPBF16 = ml_dtypes.bfloat16


def _build_program():
    nc = bacc.Bacc("TRN2", target_bir_lowering=False, debug=False, num_devices=NCORES)

    # ---- per-core external inputs ----
    xT = nc.dram_tensor("xT", [D, R], BF16, kind="ExternalInput")
    xres = nc.dram_tensor("xres", [DT, P, RB], F32, kind="ExternalInput")
    wqkv = nc.dram_tensor("wqkv", [P, NF, DT, P], BF16, kind="ExternalInput")
    # wproj reordered host-side: [p, do, hh, j, c] with dt = 2*j + hh
    wproj = nc.dram_tensor("wproj", [P, DT, HPC, NCORES, P], BF16,
                           kind="ExternalInput")
    wgate = nc.dram_tensor("wgate", [P, FGT, DT, P], BF16, kind="ExternalInput")
    wval = nc.dram_tensor("wval", [P, FGT, DT, P], BF16, kind="ExternalInput")
    wmlp = nc.dram_tensor("wmlp", [P, DT, FGT, P], BF16, kind="ExternalInput")
    cosT = nc.dram_tensor("cosT", [P, T], F32, kind="ExternalInput")
    sinT = nc.dram_tensor("sinT", [P, T], F32, kind="ExternalInput")
    rotPT = nc.dram_tensor("rotPT", [P, P], F32, kind="ExternalInput")
    masks = nc.dram_tensor("masks", [P, QBB, RB], F32, kind="ExternalInput")
    ones_in = nc.dram_tensor("ones_in", [P, P], F32, kind="ExternalInput")
    ones_bf = nc.dram_tensor("ones_bf", [P, P], BF16, kind="ExternalInput")
    ident_bf = nc.dram_tensor("ident_bf", [P, P], BF16, kind="ExternalInput")

    outT = nc.dram_tensor("outT", [DT, P, RB], F32, kind="ExternalOutput")

    # ---- internal DRAM scratch ----
    qk_dram = nc.dram_tensor("qk_dram", [2 * HPC, P, R], BF16)  # q0,q1,k0,k1
    v_dram = nc.dram_tensor("v_dram", [HPC, P, R], BF16)        # v0,v1
    a2a_in = [nc.dram_tensor(f"a2a_in{h}", [NCORES, P, RB], BF16) for h in range(HPC)]
    a2a_out = [nc.dram_tensor(f"a2a_out{h}", [NCORES, P, RB], BF16) for h in range(HPC)]

    gp_add = nc.gpsimd.tensor_tensor if GP_ADDS else nc.vector.tensor_tensor

    with tile.TileContext(nc) as tc:
        with (
            tc.tile_pool(name="const", bufs=1) as cpool,
            tc.tile_pool(name="pkv0", bufs=1) as pkv0,
        ):
            rot_t = cpool.tile([P, P], F32R)
            ones_t = cpool.tile([P, P], F32R)
            onesb_t = cpool.tile([P, P], BF16)
            id_t = cpool.tile([P, P], BF16)
            nc.sync.dma_start(rot_t[:], rotPT[:, :].bitcast(F32R))
            nc.sync.dma_start(ones_t[:], ones_in[:, :].bitcast(F32R))
            nc.sync.dma_start(onesb_t[:], ones_bf[:, :])
            nc.sync.dma_start(id_t[:], ident_bf[:, :])

            # ============ Phase 1: qkv on raw x + rope, rms applied at output ==
            with (
                tc.tile_pool(name="p1c", bufs=1) as p1c,
                tc.tile_pool(name="p1w", bufs=1) as p1w,
                tc.tile_pool(name="p1x", bufs=3) as p1x,
                tc.tile_pool(name="p1t", bufs=2) as p1t,
                tc.tile_pool(name="p1sq", bufs=4) as p1sq,
                tc.tile_pool(name="p1sa", bufs=10) as p1sa,
                tc.tile_pool(name="p1ps_ss", bufs=2, space="PSUM") as p1ps_ss,
                tc.tile_pool(name="p1ps_mm", bufs=3, space="PSUM") as p1ps_mm,
                tc.tile_pool(name="p1ps_rot", bufs=2, space="PSUM") as p1ps_rot,
            ):
                wq_t = p1w.tile([P, NF, DT, P], BF16)
                xTr = xT.rearrange("(dt p) r -> p dt r", p=P)
                cos_t = p1c.tile([P, T], F32)
                sin_t = p1c.tile([P, T], F32)
                NRB = R // RB

                kb0 = pkv0.tile([P, T], BF16)
                vb0 = pkv0.tile([P, T], BF16)
                for rb in range(NRB):
                    t0 = (rb % QBB) * RB
                    xb = p1x.tile([P, DT, RB], BF16, tag="xblk")
                    nc.sync.dma_start(xb[:], xTr[:, :, rb * RB : (rb + 1) * RB])
                    if rb == 0:
                        # after xb0 on the FIFO queue: weights per-f (so f=0
                        # matmuls start early), then rope tables
                        for f in range(NF):
                            nc.sync.dma_start(wq_t[:, f], wqkv[:, f, :, :])
                        nc.sync.dma_start(cos_t[:], cosT[:, :])
                        nc.sync.dma_start(sin_t[:], sinT[:, :])
                    # rms: squares on ACT, partial-chain adds (4 sq each) on
                    # GpSimd, partials reduced via PSUM-accumulated ones-matmuls
                    ss_ps = p1ps_ss.tile([P, RB], F32, tag="ss")
                    for pp in range(4):
                        sp = p1sa.tile([P, RB], F32R, tag="sacc")
                        for k in range(4):
                            dt = pp * 4 + k
                            sq = p1sq.tile([P, RB], F32R, tag="sq")
                            nc.scalar.activation(sq[:], xb[:, dt], SQUARE)
                            if k == 0:
                                first = sq
                            elif k == 1:
                                gp_add(sp[:], first[:], sq[:], ADD)
                            else:
                                gp_add(sp[:], sp[:], sq[:], ADD)
                        nc.tensor.matmul(ss_ps[:], ones_t[:], sp[:],
                                         start=(pp == 0), stop=(pp == 3))
                    rms = p1t.tile([P, RB], F32, tag="rms")
                    nc.vector.tensor_scalar(rms[:], ss_ps[:], 1.0 / D, EPS, MULT, ADD)
                    nc.scalar.sqrt(rms[:], rms[:])
                    rmsr = p1t.tile([P, RB], F32, tag="rmsr")
                    nc.vector.reciprocal_approx_fast(rmsr[:], rms[:])
                    # qkv matmuls on RAW x; f: 0,1=q; 2,3=k; 4,5=v
                    for f in range(NF):
                        ps = p1ps_mm.tile([P, RB], F32, tag="qkvps")
                        for dt in range(DT):
                            nc.tensor.matmul(
                                ps[:], wq_t[:, f, dt], xb[:, dt],
                                start=(dt == 0), stop=(dt == DT - 1),
                            )
                        if f < 2 * HPC:
                            raw = p1t.tile([P, RB], F32R, tag="raw")
                            nc.vector.tensor_copy(raw[:], ps[:])
                            rps = p1ps_rot.tile([P, RB], F32, tag="rotps")
                            nc.tensor.matmul(rps[:], rot_t[:], raw[:],
                                             start=True, stop=True)
                            m1 = p1t.tile([P, RB], F32, tag="m1")
                            nc.vector.tensor_tensor(
                                m1[:], ps[:], cos_t[:, t0 : t0 + RB], MULT)
                            m2 = p1t.tile([P, RB], F32, tag="m2")
                            nc.vector.tensor_tensor(
                                m2[:], rps[:], sin_t[:, t0 : t0 + RB], MULT)
                            rr = p1t.tile([P, RB], F32, tag="rr")
                            nc.vector.tensor_tensor(rr[:], m1[:], m2[:], ADD)
                            if f == 2 and rb < QBB:
                                dst = kb0[:, rb * RB : (rb + 1) * RB]
                                nc.vector.tensor_tensor(dst, rr[:], rmsr[:], MULT)
                                nc.sync.dma_start(
                                    qk_dram[f, :, rb * RB : (rb + 1) * RB], dst)
                            else:
                                out_t = p1t.tile([P, RB], BF16, tag="outt")
                                nc.vector.tensor_tensor(out_t[:], rr[:], rmsr[:],
                                                        MULT)
                                nc.sync.dma_start(
                                    qk_dram[f, :, rb * RB : (rb + 1) * RB],
                                    out_t[:])
                        else:
                            if f == 4 and rb < QBB:
                                dst = vb0[:, rb * RB : (rb + 1) * RB]
                                nc.vector.tensor_tensor(dst, ps[:], rmsr[:], MULT)
                                nc.sync.dma_start(
                                    v_dram[0, :, rb * RB : (rb + 1) * RB], dst)
                            else:
                                outv = p1t.tile([P, RB], BF16, tag="outv")
                                nc.vector.tensor_tensor(outv[:], ps[:], rmsr[:],
                                                        MULT)
                                nc.sync.dma_start(
                                    v_dram[f - 2 * HPC, :,
                                           rb * RB : (rb + 1) * RB], outv[:])

            # ============ Phase 2: attention, h outer (A2A per head) ============
            # The h0 half of proj lives inside the attention scope so its
            # matmuls fill the A2A[h1] window. PSUM: tp1+s2+o2+l1+psA2 = 8.
            with tc.tile_pool(name="p46", bufs=1) as p46:
              x2b = p46.tile([P, DT, RB], BF16)
              rms2r = p46.tile([P, RB], F32)
              with tc.tile_pool(name="px2a", bufs=1) as px2a:
                x2a = px2a.tile([P, DT, RB], F32)
                with (
                    tc.tile_pool(name="p2c", bufs=1) as p2c,
                    tc.tile_pool(name="p2kv", bufs=2) as p2kv,
                    tc.tile_pool(name="p2a", bufs=3) as p2a,
                    tc.tile_pool(name="p2t", bufs=3) as p2t,
                    tc.tile_pool(name="p4a0", bufs=1) as p4a0,
                    tc.tile_pool(name="p4w0", bufs=3) as p4w0,
                    tc.tile_pool(name="p2ps_tp", bufs=2, space="PSUM") as p2ps_tp,
                    tc.tile_pool(name="p2ps_s", bufs=2, space="PSUM") as p2ps_s,
                    tc.tile_pool(name="p2ps_o", bufs=2, space="PSUM") as p2ps_o,
                    tc.tile_pool(name="p2ps_l", bufs=1, space="PSUM") as p2ps_l,
                    tc.tile_pool(name="p4psA", bufs=1, space="PSUM") as p4psA,
                ):
                    pairs = [(h, b) for h in range(HPC) for b in range(B)]

                    def load_kv(h, b):
                        kT = p2kv.tile([P, T], BF16, tag="kT", name=f"kT{h}{b}")
                        vT = p2kv.tile([P, T], BF16, tag="vT", name=f"vT{h}{b}")
                        nc.sync.dma_start(
                            kT[:], qk_dram[HPC + h, :, b * T : (b + 1) * T])
                        nc.sync.dma_start(vT[:], v_dram[h, :, b * T : (b + 1) * T])
                        return kT, vT

                    def transpose_v(vT, h, b):
                        v_rm = p2kv.tile([P, KTB, P], BF16, tag="v_rm",
                                         name=f"vrm{h}{b}")
                        for kt in range(KTB):
                            tps = p2ps_tp.tile([P, P], BF16, tag="vtp")
                            nc.tensor.transpose(
                                tps[:], vT[:, kt * P : (kt + 1) * P], id_t[:])
                            nc.vector.tensor_copy(v_rm[:, kt], tps[:])
                        return v_rm

                    kv = (kb0, vb0)
                    mask_t = p2c.tile([P, QBB, RB], F32)
                    nc.sync.dma_start(mask_t[:], masks[:, :, :])
                    for pi, (h, b) in enumerate(pairs):
                        kT, vT = kv
                        v_rm = transpose_v(vT, h, b)
                        kv_next = (load_kv(*pairs[pi + 1])
                                   if pi + 1 < len(pairs) else None)
                        for qb in range(QBB):
                            qTs = p2t.tile([P, RB], BF16, tag="qTs")
                            nc.sync.dma_start(
                                qTs[:],
                                qk_dram[h, :, b * T + qb * RB :
                                        b * T + (qb + 1) * RB])
                            nkt = 4 * qb + 4
                            at = p2a.tile([P, KTB, RB], BF16, tag="at")
                            o_ps = p2ps_o.tile([P, RB], F32, tag="ops")
                            l_ps = p2ps_l.tile([P, RB], F32, tag="lps")
                            for kt in range(nkt):
                                s_ps = p2ps_s.tile([P, RB], F32, tag="sps")
                                nc.tensor.matmul(
                                    s_ps[:], kT[:, kt * P : (kt + 1) * P], qTs[:],
                                    start=True, stop=True)
                                if kt >= 4 * qb:
                                    msk = p2t.tile([P, RB], F32, tag="msk")
                                    nc.vector.tensor_tensor(
                                        msk[:], s_ps[:], mask_t[:, kt - 4 * qb],
                                        ADD)
                                    esrc = msk
                                else:
                                    esrc = s_ps
                                nc.scalar.activation(at[:, kt], esrc[:], EXP,
                                                     scale=SCALE)
                                nc.tensor.matmul(
                                    o_ps[:], v_rm[:, kt], at[:, kt],
                                    start=(kt == 0), stop=(kt == nkt - 1))
                                nc.tensor.matmul(
                                    l_ps[:], onesb_t[:], at[:, kt],
                                    start=(kt == 0), stop=(kt == nkt - 1))
                            rl = p2t.tile([P, RB], F32, tag="rl")
                            nc.vector.reciprocal_approx_fast(rl[:], l_ps[:])
                            ot = p2t.tile([P, RB], BF16, tag="ot")
                            nc.vector.tensor_tensor(ot[:], o_ps[:], rl[:], MULT)
                            j = b * QBB + qb
                            last_ot = nc.sync.dma_start(a2a_in[h][j, :, :], ot[:])
                        kv = kv_next
                        if b == B - 1:
                            nc.gpsimd.collective_compute(
                                "AllToAll", mybir.AluOpType.bypass,
                                ins=[a2a_in[h][:, :, :]],
                                outs=[a2a_out[h][:, :, :]],
                                replica_groups=[list(range(NCORES))])

                    # residual x slice (f32): DMA'd straight into x2a,
                    # queued after the attention DMAs
                    xresr = xres.rearrange("dt p rb -> p dt rb")
                    nc.sync.dma_start(x2a[:], xresr[:, :, :])

                    # ---- proj h0 pass: consumes a2a_out[0]; overlaps late
                    # attention + A2A[h1]
                    otf0 = p4a0.tile([P, NCORES, RB], BF16)
                    for j in range(NCORES):
                        ld = nc.sync.dma_start(otf0[:, j], a2a_out[0][j, :, :])
                        add_dep_helper(ld.ins, last_ot.ins, False,
                                       "A2A-gated load must not head-block "
                                       "attention DMAs on the sync FIFO")
                    for do in range(DT):
                        wpA = p4w0.tile([P, NCORES, P], BF16, tag="wp")
                        nc.sync.dma_start(wpA[:], wproj[:, do, 0, :, :])
                        psA = p4psA.tile([P, RB], F32, tag="ppsA")
                        for j in range(NCORES):
                            nc.tensor.matmul(psA[:], wpA[:, j], otf0[:, j],
                                             start=(j == 0),
                                             stop=(j == NCORES - 1))
                        nc.vector.tensor_tensor(x2a[:, do], psA[:],
                                                x2a[:, do], ADD)

                # ========= Phase 4b: proj h1 pass + norm2 ======================
                with (
                    tc.tile_pool(name="p4i", bufs=1) as p4i,
                    tc.tile_pool(name="p4w", bufs=3) as p4w,
                    tc.tile_pool(name="p4t", bufs=2) as p4t,
                    tc.tile_pool(name="p4sq", bufs=4) as p4sq,
                    tc.tile_pool(name="p4sa", bufs=6) as p4sa,
                    tc.tile_pool(name="p4psB", bufs=2, space="PSUM") as p4psB,
                    tc.tile_pool(name="p45ps_ss", bufs=1, space="PSUM") as p45ss,
                ):
                    otf1 = p4i.tile([P, NCORES, RB], BF16)
                    for j in range(NCORES):
                        nc.scalar.dma_start(otf1[:, j], a2a_out[1][j, :, :])
                    ss2 = p45ss.tile([P, RB], F32, tag="ss2")
                    sps = []
                    for do in range(DT):
                        wpB = p4w.tile([P, NCORES, P], BF16, tag="wp")
                        nc.sync.dma_start(wpB[:], wproj[:, do, 1, :, :])
                        psB = p4psB.tile([P, RB], F32, tag="ppsB")
                        for j in range(NCORES):
                            nc.tensor.matmul(psB[:], wpB[:, j], otf1[:, j],
                                             start=(j == 0),
                                             stop=(j == NCORES - 1))
                        nc.vector.tensor_tensor(x2b[:, do], x2a[:, do], psB[:],
                                                ADD)
                        sq = p4sq.tile([P, RB], F32R, tag="sq2")
                        nc.scalar.activation(sq[:], x2b[:, do], SQUARE)
                        k = do % 4
                        if k == 0:
                            first = sq
                        elif k == 1:
                            sp = p4sa.tile([P, RB], F32R, tag="sacc2")
                            gp_add(sp[:], first[:], sq[:], ADD)
                            sps.append(sp)
                        else:
                            gp_add(sps[-1][:], sps[-1][:], sq[:], ADD)
                        if k == 3:
                            pp = do // 4
                            nc.tensor.matmul(ss2[:], ones_t[:], sps[-1][:],
                                             start=(pp == 0), stop=(pp == 3))
                    rms2 = p4t.tile([P, RB], F32, tag="rms2")
                    nc.vector.tensor_scalar(rms2[:], ss2[:], 1.0 / D, EPS, MULT,
                                            ADD)
                    nc.scalar.sqrt(rms2[:], rms2[:])
                    nc.vector.reciprocal_approx_fast(rms2r[:], rms2[:])

              # ============ Phase 6: SwiGLU MLP (fg quarters) ============
              # gate/val matmuls consume un-normalized x2b; rms applied to the
              # silu input and once to the final output (row-scale commutes).
              quarters = [(0, 11), (11, 22), (22, 33), (33, FGT)]
              with (
                  tc.tile_pool(name="p6a", bufs=1) as p6a,
                  tc.tile_pool(name="p6g", bufs=1) as p6g,
                  tc.tile_pool(name="p6w", bufs=3) as p6w,
                  tc.tile_pool(name="p6t", bufs=3) as p6t,
                  tc.tile_pool(name="p6ps_g", bufs=2, space="PSUM") as p6ps_g,
                  tc.tile_pool(name="p6ps_v", bufs=2, space="PSUM") as p6ps_v,
                  tc.tile_pool(name="p6ps_o", bufs=1, space="PSUM") as p6ps_o,
              ):
                  out_acc = p6a.tile([P, DT, RB], F32)
                  for qi, (fg0, fg1) in enumerate(quarters):
                      nq = fg1 - fg0
                      gt = p6g.tile([P, 11, RB], BF16, tag="gt")
                      for fi in range(nq):
                          fg = fg0 + fi
                          wg = p6w.tile([P, DT, P], BF16, tag="wg")
                          nc.sync.dma_start(wg[:], wgate[:, fg, :, :])
                          wv = p6w.tile([P, DT, P], BF16, tag="wv")
                          nc.sync.dma_start(wv[:], wval[:, fg, :, :])
                          g_ps = p6ps_g.tile([P, RB], F32, tag="gps")
                          for dt in range(DT):
                              nc.tensor.matmul(g_ps[:], wg[:, dt], x2b[:, dt],
                                               start=(dt == 0),
                                               stop=(dt == DT - 1))
                          v_ps = p6ps_v.tile([P, RB], F32, tag="vps")
                          for dt in range(DT):
                              nc.tensor.matmul(v_ps[:], wv[:, dt], x2b[:, dt],
                                               start=(dt == 0),
                                               stop=(dt == DT - 1))
                          gn = p6t.tile([P, RB], F32, tag="gn")
                          nc.vector.tensor_tensor(gn[:], g_ps[:], rms2r[:], MULT)
                          sg = p6t.tile([P, RB], F32, tag="sg")
                          if SIM_SILU:
                              nc.scalar.activation(
                                  sg[:], gn[:],
                                  mybir.ActivationFunctionType.Sigmoid)
                              nc.vector.tensor_tensor(sg[:], sg[:], gn[:], MULT)
                          else:
                              nc.scalar.activation(sg[:], gn[:], SILU)
                          nc.vector.tensor_tensor(gt[:, fi], sg[:], v_ps[:], MULT)
                      for do in range(DT):
                          wm = p6w.tile([P, 11, P], BF16, tag="wm")
                          nc.sync.dma_start(wm[:, :nq], wmlp[:, do, fg0:fg1, :])
                          o_ps = p6ps_o.tile([P, RB], F32, tag="ops6")
                          for fi in range(nq):
                              nc.tensor.matmul(o_ps[:], wm[:, fi], gt[:, fi],
                                               start=(fi == 0),
                                               stop=(fi == nq - 1))
                          if qi == 0:
                              nc.vector.tensor_copy(out_acc[:, do], o_ps[:])
                          elif qi < len(quarters) - 1:
                              nc.vector.tensor_tensor(
                                  out_acc[:, do], o_ps[:], out_acc[:, do], ADD)
                          else:
                              tmp = p6t.tile([P, RB], F32, tag="tmp6")
                              nc.vector.tensor_tensor(
                                  tmp[:], o_ps[:], out_acc[:, do], ADD)
                              fin = p6t.tile([P, RB], F32, tag="fin")
                              nc.vector.tensor_tensor(fin[:], tmp[:], rms2r[:],
                                                      MULT)
                              nc.vector.tensor_tensor(fin[:], fin[:], x2b[:, do],
                                                      ADD)
                              nc.sync.dma_start(outT[do, :, :], fin[:])

    nc.compile()
    return nc


def _rope_tables():
    inv_freq = 1.0 / (ROPE_BASE ** (np.arange(0, DH, 2, dtype=np.float32) / DH))
    t = np.arange(T, dtype=np.float32)
    freqs = np.outer(t, inv_freq)
    emb = np.repeat(freqs, 2, axis=-1)  # [T, DH]
    return np.cos(emb).astype(np.float32), np.sin(emb).astype(np.float32)


def _tile4(w, n_out_tiles, n_in_tiles):
    """[F_out, D_in] -> [P(p of d-tile), F_out/P tiles, D_in/P tiles, P(c of f-tile)].

    Element [p, f, dt, c] = w[f*P + c, dt*P + p].
    """
    Fo, Di = w.shape
    assert Fo == n_out_tiles * P and Di == n_in_tiles * P
    v = w.reshape(n_out_tiles, P, n_in_tiles, P)
    return np.ascontiguousarray(v.transpose(3, 0, 2, 1))


def _prepare_inputs(x, norm1_w, norm2_w, c_attn_w, c_proj_w, c_gate_w, c_val_w,
                    c_mlp_proj_w):
    xf = np.ascontiguousarray(x.reshape(R, D).T)  # [D, R] f32
    xf_bf = xf.astype(NPBF16)
    cos, sin = _rope_tables()
    cosT = np.ascontiguousarray(cos.T)  # [DH, T]
    sinT = np.ascontiguousarray(sin.T)

    # rot-half signed permutation: (P @ q)[d] = -q[d+1] (d even), q[d-1] (d odd)
    rotP = np.zeros((P, P), np.float32)
    for d in range(0, P, 2):
        rotP[d, d + 1] = -1.0
        rotP[d + 1, d] = 1.0
    rotPT = np.ascontiguousarray(rotP.T)

    # additive causal masks for diagonal k-tiles, ST layout [k partition, q col]
    masks = np.zeros((P, QBB, RB), np.float32)
    for di in range(QBB):
        p_idx = np.arange(P)[:, None] + di * P
        c_idx = np.arange(RB)[None, :]
        masks[:, di, :] = np.where(p_idx <= c_idx, 0.0, NEG)

    ones_in = np.ones((P, P), np.float32)
    ones_bf = np.ones((P, P), NPBF16)
    ident_bf = np.eye(P).astype(NPBF16)

    w1 = norm1_w.astype(np.float32)
    w2 = norm2_w.astype(np.float32)
    attn_w = c_attn_w.astype(np.float32) * w1[None, :]     # fold norm1
    gate_w = c_gate_w.astype(np.float32) * w2[None, :]     # fold norm2
    val_w = c_val_w.astype(np.float32) * w2[None, :]

    gate_p = np.zeros((FGP, D), np.float32)
    gate_p[:FG] = gate_w
    val_p = np.zeros((FGP, D), np.float32)
    val_p[:FG] = val_w
    mlp_p = np.zeros((D, FGP), np.float32)
    mlp_p[:, :FG] = c_mlp_proj_w.astype(np.float32)

    # wproj: [p, do, dt, c] -> [p, do, hh, j, c] with dt = 2*j + hh
    wproj_t = _tile4(c_proj_w.astype(np.float32), DT, DT)
    wproj_r = np.ascontiguousarray(
        wproj_t.reshape(P, DT, NCORES, HPC, P).transpose(0, 1, 3, 2, 4)
    ).astype(NPBF16)
    wgate_t = _tile4(gate_p, FGT, DT).astype(NPBF16)
    wval_t = _tile4(val_p, FGT, DT).astype(NPBF16)
    # wmlp: lhsT [fg partition, dout col]: [p, do, fg, c] = mlp_p[do*P+c, fg*P+p]
    wmlp_t = np.ascontiguousarray(
        mlp_p.reshape(DT, P, FGT, P).transpose(3, 0, 2, 1)
    ).astype(NPBF16)

    in_maps = []
    for i in range(NCORES):
        h0, h1 = 2 * i, 2 * i + 1
        rows = []
        for base in (0, D, 2 * D):  # q, k, v row groups of c_attn_w
            rows.extend(range(base + h0 * DH, base + h0 * DH + DH))
            rows.extend(range(base + h1 * DH, base + h1 * DH + DH))
        wsel = attn_w[rows, :]                       # [768, D]
        wqkv_t = _tile4(wsel, NF, DT).astype(NPBF16)
        xres_i = np.ascontiguousarray(
            xf[:, i * RB : (i + 1) * RB].reshape(DT, P, RB)
        )
        in_maps.append({
            "xT": xf_bf,
            "xres": xres_i,
            "wqkv": wqkv_t,
            "wproj": wproj_r,
            "wgate": wgate_t,
            "wval": wval_t,
            "wmlp": wmlp_t,
            "cosT": cosT,
            "sinT": sinT,
            "rotPT": rotPT,
            "masks": masks,
            "ones_in": ones_in,
            "ones_bf": ones_bf,
            "ident_bf": ident_bf,
        })
    return in_maps


_NC_CACHE = None


def _get_program():
    global _NC_CACHE
    if _NC_CACHE is None:
        _NC_CACHE = _build_program()
    return _NC_CACHE


def run(inputs, trace=False):
    """Returns (output [B,T,D], exec_time_ns or None)."""
    in_maps = _prepare_inputs(**inputs)
    nc = _get_program()
    res = run_bass_kernel_spmd(nc, in_maps, list(range(NCORES)), trace=trace)
    cols = []
    for i in range(NCORES):
        o = res.results[i]["outT"]          # [DT, P, RB]
        cols.append(o.reshape(D, RB))
    full_T = np.concatenate(cols, axis=1)   # [D, R]
    out = np.ascontiguousarray(full_T.T).reshape(B, T, D).astype(np.float32)
    return out, res.exec_time_ns


def kernel(**inputs) -> np.ndarray:
    out, _ = run(inputs, trace=False)
    return out


# revision 21
# speedup vs baseline: 1.1547x; 1.0007x over previous
"""Trainium2 Bass kernel for a dense pre-norm transformer block (B=2, T=2048,
D=2048, H=16, DH=128, FG=5461, SwiGLU MLP, RoPE, causal attention).

Sharding: tensor-parallel attention over heads (2 heads/core on 8 cores), one
AllToAll per head to reshard to 512 query columns per core, then fully local
proj + MLP per core (weights replicated, streamed from HBM).

All on-device activations are kept transposed ([feature, row]) so every
matmul is lhsT(=weight tile).T @ rhs(=activationT tile) with the contraction
dim on SBUF partitions.

v3: all big matmuls in bf16 (incl. QK^T); phase-1 qkv matmuls use 1024-wide
moving operands; A2A[h1] hidden under the h0 half of proj; approx
reciprocals; norm2 rms factored out of the MLP; rms partial-sum adds on
GpSimd (phase 1) / DVE (phase 4); residual kept in SBUF.
"""

import numpy as np
import ml_dtypes

import concourse.bass as bass
import concourse.mybir as mybir
import concourse.tile as tile
from concourse.tile import add_dep_helper
from concourse import bacc
from concourse.bass_utils import run_bass_kernel_spmd

# Problem constants
B, T, D = 2, 2048, 2048
H, DH = 16, 128
FG = 5461
EPS = 1e-5
ROPE_BASE = 10000.0

P = 128
NCORES = 8
R = B * T                    # 4096 rows total
RB = 512                     # rows per core / per-core q-cols
RB1 = 1024                   # phase-1 row block (bf16 moving max)
NRB1 = R // RB1              # 4 phase-1 row blocks
DT = D // P                  # 16 d-tiles
HPC = H // NCORES            # 2 heads per core
NF = 3 * HPC                 # 6 feature tiles per core in qkv (q0,q1,k0,k1,v0,v1)
FGP = 5504                   # FG padded to 43*128
FGT = FGP // P               # 43 fg tiles
KTB = T // P                 # 16 k-tiles per batch
QBB = T // RB                # 4 q-blocks per batch
NEG = -1.0e30
SCALE = 1.0 / np.sqrt(DH)

SIM_SILU = False  # replace Silu with sigmoid+mults (for CoreSim, which lacks Silu)
GP_ADDS = True    # run phase-1 rms partial adds on GpSimd (else DVE)

F32 = mybir.dt.float32
F32R = mybir.dt.float32r
BF16 = mybir.dt.bfloat16
EXP = mybir.ActivationFunctionType.Exp
SQUARE = mybir.ActivationFunctionType.Square
SILU = mybir.ActivationFunctionType.Silu
MULT = mybir.AluOpType.mult
ADD = mybir.AluOpType.add

NPBF16 = ml_dtypes.bfloat16


def _build_program():
    nc = bacc.Bacc("TRN2", target_bir_lowering=False, debug=False, num_devices=NCORES)

    # ---- per-core external inputs ----
    xT = nc.dram_tensor("xT", [D, R], BF16, kind="ExternalInput")
    xres = nc.dram_tensor("xres", [DT, P, RB], F32, kind="ExternalInput")
    wqkv = nc.dram_tensor("wqkv", [P, NF, DT, P], BF16, kind="ExternalInput")
    # wproj reordered host-side: [p, do, hh, j, c] with dt = 2*j + hh
    wproj = nc.dram_tensor("wproj", [P, DT, HPC, NCORES, P], BF16,
                           kind="ExternalInput")
    wgate = nc.dram_tensor("wgate", [P, FGT, DT, P], BF16, kind="ExternalInput")
    wval = nc.dram_tensor("wval", [P, FGT, DT, P], BF16, kind="ExternalInput")
    wmlp = nc.dram_tensor("wmlp", [P, DT, FGT, P], BF16, kind="ExternalInput")
    cosT = nc.dram_tensor("cosT", [P, T], F32, kind="ExternalInput")
    sinT = nc.dram_tensor("sinT", [P, T], F32, kind="ExternalInput")
    rotPT = nc.dram_tensor("rotPT", [P, P], F32, kind="ExternalInput")
    masks = nc.dram_tensor("masks", [P, QBB, RB], F32, kind="ExternalInput")
    ones_in = nc.dram_tensor("ones_in", [P, P], F32, kind="ExternalInput")
    ones_bf = nc.dram_tensor("ones_bf", [P, P], BF16, kind="ExternalInput")
    ident_bf = nc.dram_tensor("ident_bf", [P, P], BF16, kind="ExternalInput")

    outT = nc.dram_tensor("outT", [DT, P, RB], F32, kind="ExternalOutput")

    # ---- internal DRAM scratch ----
    qk_dram = nc.dram_tensor("qk_dram", [2 * HPC, P, R], BF16)  # q0,q1,k0,k1
    v_dram = nc.dram_tensor("v_dram", [HPC, P, R], BF16)        # v0,v1
    a2a_in = [nc.dram_tensor(f"a2a_in{h}", [NCORES, P, RB], BF16) for h in range(HPC)]
    a2a_out = [nc.dram_tensor(f"a2a_out{h}", [NCORES, P, RB], BF16) for h in range(HPC)]

    gp_add = nc.gpsimd.tensor_tensor if GP_ADDS else nc.vector.tensor_tensor

    with tile.TileContext(nc) as tc:
        with (
            tc.tile_pool(name="const", bufs=1) as cpool,
            tc.tile_pool(name="pkv0", bufs=1) as pkv0,
        ):
            rot_t = cpool.tile([P, P], F32R)
            ones_t = cpool.tile([P, P], F32R)
            onesb_t = cpool.tile([P, P], BF16)
            id_t = cpool.tile([P, P], BF16)
            nc.sync.dma_start(rot_t[:], rotPT[:, :].bitcast(F32R))
            nc.sync.dma_start(ones_t[:], ones_in[:, :].bitcast(F32R))
            nc.sync.dma_start(onesb_t[:], ones_bf[:, :])
            nc.sync.dma_start(id_t[:], ident_bf[:, :])

            # ============ Phase 1: qkv on raw x + rope, rms applied at output ==
            with (
                tc.tile_pool(name="p1c", bufs=1) as p1c,
                tc.tile_pool(name="p1w", bufs=1) as p1w,
                tc.tile_pool(name="p1x", bufs=3) as p1x,
                tc.tile_pool(name="p1t", bufs=2) as p1t,
                tc.tile_pool(name="p1sq", bufs=4) as p1sq,
                tc.tile_pool(name="p1sa", bufs=10) as p1sa,
                tc.tile_pool(name="p1ps_ss", bufs=2, space="PSUM") as p1ps_ss,
                tc.tile_pool(name="p1ps_mm", bufs=3, space="PSUM") as p1ps_mm,
                tc.tile_pool(name="p1ps_rot", bufs=2, space="PSUM") as p1ps_rot,
            ):
                wq_t = p1w.tile([P, NF, DT, P], BF16)
                xTr = xT.rearrange("(dt p) r -> p dt r", p=P)
                cos_t = p1c.tile([P, T], F32)
                sin_t = p1c.tile([P, T], F32)
                NRB = R // RB

                kb0 = pkv0.tile([P, T], BF16)
                vb0 = pkv0.tile([P, T], BF16)
                for rb in range(NRB):
                    t0 = (rb % QBB) * RB
                    xb = p1x.tile([P, DT, RB], BF16, tag="xblk")
                    nc.sync.dma_start(xb[:], xTr[:, :, rb * RB : (rb + 1) * RB])
                    if rb == 0:
                        # after xb0 on the FIFO queue: weights per-f (so f=0
                        # matmuls start early), then rope tables
                        for f in range(NF):
                            nc.sync.dma_start(wq_t[:, f], wqkv[:, f, :, :])
                        nc.sync.dma_start(cos_t[:], cosT[:, :])
                        nc.sync.dma_start(sin_t[:], sinT[:, :])
                    # rms: squares on ACT, partial-chain adds (4 sq each) on
                    # GpSimd, partials reduced via PSUM-accumulated ones-matmuls
                    ss_ps = p1ps_ss.tile([P, RB], F32, tag="ss")
                    for pp in range(4):
                        sp = p1sa.tile([P, RB], F32R, tag="sacc")
                        for k in range(4):
                            dt = pp * 4 + k
                            sq = p1sq.tile([P, RB], F32R, tag="sq")
                            nc.scalar.activation(sq[:], xb[:, dt], SQUARE)
                            if k == 0:
                                first = sq
                            elif k == 1:
                                gp_add(sp[:], first[:], sq[:], ADD)
                            else:
                                gp_add(sp[:], sp[:], sq[:], ADD)
                        nc.tensor.matmul(ss_ps[:], ones_t[:], sp[:],
                                         start=(pp == 0), stop=(pp == 3))
                    rms = p1t.tile([P, RB], F32, tag="rms")
                    nc.vector.tensor_scalar(rms[:], ss_ps[:], 1.0 / D, EPS, MULT, ADD)
                    nc.scalar.sqrt(rms[:], rms[:])
                    rmsr = p1t.tile([P, RB], F32, tag="rmsr")
                    nc.vector.reciprocal_approx_fast(rmsr[:], rms[:])
                    # qkv matmuls on RAW x; f: 0,1=q; 2,3=k; 4,5=v
                    for f in range(NF):
                        ps = p1ps_mm.tile([P, RB], F32, tag="qkvps")
                        for dt in range(DT):
                            nc.tensor.matmul(
                                ps[:], wq_t[:, f, dt], xb[:, dt],
                                start=(dt == 0), stop=(dt == DT - 1),
                            )
                        if f < 2 * HPC:
                            raw = p1t.tile([P, RB], F32R, tag="raw")
                            nc.vector.tensor_copy(raw[:], ps[:])
                            rps = p1ps_rot.tile([P, RB], F32, tag="rotps")
                            nc.tensor.matmul(rps[:], rot_t[:], raw[:],
                                             start=True, stop=True)
                            m1 = p1t.tile([P, RB], F32, tag="m1")
                            nc.vector.tensor_tensor(
                                m1[:], ps[:], cos_t[:, t0 : t0 + RB], MULT)
                            m2 = p1t.tile([P, RB], F32, tag="m2")
                            nc.vector.tensor_tensor(
                                m2[:], rps[:], sin_t[:, t0 : t0 + RB], MULT)
                            rr = p1t.tile([P, RB], F32, tag="rr")
                            nc.vector.tensor_tensor(rr[:], m1[:], m2[:], ADD)
                            if f == 2 and rb < QBB:
                                dst = kb0[:, rb * RB : (rb + 1) * RB]
                                nc.vector.tensor_tensor(dst, rr[:], rmsr[:], MULT)
                                nc.sync.dma_start(
                                    qk_dram[f, :, rb * RB : (rb + 1) * RB], dst)
                            else:
                                out_t = p1t.tile([P, RB], BF16, tag="outt")
                                nc.vector.tensor_tensor(out_t[:], rr[:], rmsr[:],
                                                        MULT)
                                nc.sync.dma_start(
                                    qk_dram[f, :, rb * RB : (rb + 1) * RB],
                                    out_t[:])
                        else:
                            if f == 4 and rb < QBB:
                                dst = vb0[:, rb * RB : (rb + 1) * RB]
                                nc.vector.tensor_tensor(dst, ps[:], rmsr[:], MULT)
                                nc.sync.dma_start(
                                    v_dram[0, :, rb * RB : (rb + 1) * RB], dst)
                            else:
                                outv = p1t.tile([P, RB], BF16, tag="outv")
                                nc.vector.tensor_tensor(outv[:], ps[:], rmsr[:],
                                                        MULT)
                                nc.sync.dma_start(
                                    v_dram[f - 2 * HPC, :,
                                           rb * RB : (rb + 1) * RB], outv[:])

            # ============ Phase 2: attention, h outer (A2A per head) ============
            # The h0 half of proj lives inside the attention scope so its
            # matmuls fill the A2A[h1] window. PSUM: tp1+s2+o2+l1+psA2 = 8.
            with tc.tile_pool(name="p46", bufs=1) as p46:
              x2b = p46.tile([P, DT, RB], BF16)
              rms2r = p46.tile([P, RB], F32)
              with tc.tile_pool(name="px2a", bufs=1) as px2a:
                x2a = px2a.tile([P, DT, RB], F32)
                with (
                    tc.tile_pool(name="p2c", bufs=1) as p2c,
                    tc.tile_pool(name="p2kv", bufs=2) as p2kv,
                    tc.tile_pool(name="p2a", bufs=3) as p2a,
                    tc.tile_pool(name="p2t", bufs=3) as p2t,
                    tc.tile_pool(name="p4a0", bufs=1) as p4a0,
                    tc.tile_pool(name="p4w0", bufs=3) as p4w0,
                    tc.tile_pool(name="p2ps_tp", bufs=2, space="PSUM") as p2ps_tp,
                    tc.tile_pool(name="p2ps_s", bufs=2, space="PSUM") as p2ps_s,
                    tc.tile_pool(name="p2ps_o", bufs=2, space="PSUM") as p2ps_o,
                    tc.tile_pool(name="p2ps_l", bufs=1, space="PSUM") as p2ps_l,
                    tc.tile_pool(name="p4psA", bufs=1, space="PSUM") as p4psA,
                ):
                    pairs = [(h, b) for h in range(HPC) for b in range(B)]

                    def load_kv(h, b):
                        kT = p2kv.tile([P, T], BF16, tag="kT", name=f"kT{h}{b}")
                        vT = p2kv.tile([P, T], BF16, tag="vT", name=f"vT{h}{b}")
                        nc.sync.dma_start(
                            kT[:], qk_dram[HPC + h, :, b * T : (b + 1) * T])
                        nc.sync.dma_start(vT[:], v_dram[h, :, b * T : (b + 1) * T])
                        return kT, vT

                    def transpose_v(vT, h, b):
                        v_rm = p2kv.tile([P, KTB, P], BF16, tag="v_rm",
                                         name=f"vrm{h}{b}")
                        for kt in range(KTB):
                            tps = p2ps_tp.tile([P, P], BF16, tag="vtp")
                            nc.tensor.transpose(
                                tps[:], vT[:, kt * P : (kt + 1) * P], id_t[:])
                            nc.vector.tensor_copy(v_rm[:, kt], tps[:])
                        return v_rm

                    kv = (kb0, vb0)
                    mask_t = p2c.tile([P, QBB, RB], F32)
                    nc.sync.dma_start(mask_t[:], masks[:, :, :])
                    for pi, (h, b) in enumerate(pairs):
                        kT, vT = kv
                        v_rm = transpose_v(vT, h, b)
                        kv_next = (load_kv(*pairs[pi + 1])
                                   if pi + 1 < len(pairs) else None)
                        for qb in range(QBB):
                            qTs = p2t.tile([P, RB], BF16, tag="qTs")
                            last_qts = nc.sync.dma_start(
                                qTs[:],
                                qk_dram[h, :, b * T + qb * RB :
                                        b * T + (qb + 1) * RB])
                            nkt = 4 * qb + 4
                            at = p2a.tile([P, KTB, RB], BF16, tag="at")
                            o_ps = p2ps_o.tile([P, RB], F32, tag="ops")
                            l_ps = p2ps_l.tile([P, RB], F32, tag="lps")
                            for kt in range(nkt):
                                s_ps = p2ps_s.tile([P, RB], F32, tag="sps")
                                nc.tensor.matmul(
                                    s_ps[:], kT[:, kt * P : (kt + 1) * P], qTs[:],
                                    start=True, stop=True)
                                if kt >= 4 * qb:
                                    msk = p2t.tile([P, RB], F32, tag="msk")
                                    nc.vector.tensor_tensor(
                                        msk[:], s_ps[:], mask_t[:, kt - 4 * qb],
                                        ADD)
                                    esrc = msk
                                else:
                                    esrc = s_ps
                                nc.scalar.activation(at[:, kt], esrc[:], EXP,
                                                     scale=SCALE)
                                nc.tensor.matmul(
                                    o_ps[:], v_rm[:, kt], at[:, kt],
                                    start=(kt == 0), stop=(kt == nkt - 1))
                                nc.tensor.matmul(
                                    l_ps[:], onesb_t[:], at[:, kt],
                                    start=(kt == 0), stop=(kt == nkt - 1))
                            rl = p2t.tile([P, RB], F32, tag="rl")
                            nc.vector.reciprocal_approx_fast(rl[:], l_ps[:])
                            ot = p2t.tile([P, RB], BF16, tag="ot")
                            nc.vector.tensor_tensor(ot[:], o_ps[:], rl[:], MULT)
                            j = b * QBB + qb
                            last_ot = nc.sync.dma_start(a2a_in[h][j, :, :], ot[:])
                        kv = kv_next
                        if b == B - 1:
                            nc.gpsimd.collective_compute(
                                "AllToAll", mybir.AluOpType.bypass,
                                ins=[a2a_in[h][:, :, :]],
                                outs=[a2a_out[h][:, :, :]],
                                replica_groups=[list(range(NCORES))])

                    # residual x slice (f32): DMA'd straight into x2a,
                    # queued after the attention DMAs
                    xresr = xres.rearrange("dt p rb -> p dt rb")
                    nc.sync.dma_start(x2a[:], xresr[:, :, :])

                    # ---- proj h0 pass: consumes a2a_out[0]; overlaps late
                    # attention + A2A[h1]
                    otf0 = p4a0.tile([P, NCORES, RB], BF16)
                    for j in range(NCORES):
                        ld = nc.sync.dma_start(otf0[:, j], a2a_out[0][j, :, :])
                        add_dep_helper(ld.ins, last_qts.ins, False,
                                       "A2A-gated load must not head-block "
                                       "attention DMAs on the sync FIFO")
                    for do in range(DT):
                        wpA = p4w0.tile([P, NCORES, P], BF16, tag="wp")
                        nc.sync.dma_start(wpA[:], wproj[:, do, 0, :, :])
                        psA = p4psA.tile([P, RB], F32, tag="ppsA")
                        for j in range(NCORES):
                            nc.tensor.matmul(psA[:], wpA[:, j], otf0[:, j],
                                             start=(j == 0),
                                             stop=(j == NCORES - 1))
                        nc.vector.tensor_tensor(x2a[:, do], psA[:],
                                                x2a[:, do], ADD)

                # ========= Phase 4b: proj h1 pass + norm2 ======================
                with (
                    tc.tile_pool(name="p4i", bufs=1) as p4i,
                    tc.tile_pool(name="p4w", bufs=3) as p4w,
                    tc.tile_pool(name="p4t", bufs=2) as p4t,
                    tc.tile_pool(name="p4sq", bufs=4) as p4sq,
                    tc.tile_pool(name="p4sa", bufs=6) as p4sa,
                    tc.tile_pool(name="p4psB", bufs=2, space="PSUM") as p4psB,
                    tc.tile_pool(name="p45ps_ss", bufs=1, space="PSUM") as p45ss,
                ):
                    otf1 = p4i.tile([P, NCORES, RB], BF16)
                    for j in range(NCORES):
                        nc.scalar.dma_start(otf1[:, j], a2a_out[1][j, :, :])
                    ss2 = p45ss.tile([P, RB], F32, tag="ss2")
                    sps = []
                    for do in range(DT):
                        wpB = p4w.tile([P, NCORES, P], BF16, tag="wp")
                        nc.sync.dma_start(wpB[:], wproj[:, do, 1, :, :])
                        psB = p4psB.tile([P, RB], F32, tag="ppsB")
                        for j in range(NCORES):
                            nc.tensor.matmul(psB[:], wpB[:, j], otf1[:, j],
                                             start=(j == 0),
                                             stop=(j == NCORES - 1))
                        nc.vector.tensor_tensor(x2b[:, do], x2a[:, do], psB[:],
                                                ADD)
                        sq = p4sq.tile([P, RB], F32R, tag="sq2")
                        nc.scalar.activation(sq[:], x2b[:, do], SQUARE)
                        k = do % 4
                        if k == 0:
                            first = sq
                        elif k == 1:
                            sp = p4sa.tile([P, RB], F32R, tag="sacc2")
                            gp_add(sp[:], first[:], sq[:], ADD)
                            sps.append(sp)
                        else:
                            gp_add(sps[-1][:], sps[-1][:], sq[:], ADD)
                        if k == 3:
                            pp = do // 4
                            nc.tensor.matmul(ss2[:], ones_t[:], sps[-1][:],
                                             start=(pp == 0), stop=(pp == 3))
                    rms2 = p4t.tile([P, RB], F32, tag="rms2")
                    nc.vector.tensor_scalar(rms2[:], ss2[:], 1.0 / D, EPS, MULT,
                                            ADD)
                    nc.scalar.sqrt(rms2[:], rms2[:])
                    nc.vector.reciprocal_approx_fast(rms2r[:], rms2[:])

              # ============ Phase 6: SwiGLU MLP (fg quarters) ============
              # gate/val matmuls consume un-normalized x2b; rms applied to the
              # silu input and once to the final output (row-scale commutes).
              quarters = [(0, 11), (11, 22), (22, 33), (33, FGT)]
              with (
                  tc.tile_pool(name="p6a", bufs=1) as p6a,
                  tc.tile_pool(name="p6g", bufs=1) as p6g,
                  tc.tile_pool(name="p6w", bufs=3) as p6w,
                  tc.tile_pool(name="p6t", bufs=3) as p6t,
                  tc.tile_pool(name="p6ps_g", bufs=2, space="PSUM") as p6ps_g,
                  tc.tile_pool(name="p6ps_v", bufs=2, space="PSUM") as p6ps_v,
                  tc.tile_pool(name="p6ps_o", bufs=1, space="PSUM") as p6ps_o,
              ):
                  out_acc = p6a.tile([P, DT, RB], F32)
                  for qi, (fg0, fg1) in enumerate(quarters):
                      nq = fg1 - fg0
                      gt = p6g.tile([P, 11, RB], BF16, tag="gt")
                      for fi in range(nq):
                          fg = fg0 + fi
                          wg = p6w.tile([P, DT, P], BF16, tag="wg")
                          nc.sync.dma_start(wg[:], wgate[:, fg, :, :])
                          wv = p6w.tile([P, DT, P], BF16, tag="wv")
                          nc.sync.dma_start(wv[:], wval[:, fg, :, :])
                          g_ps = p6ps_g.tile([P, RB], F32, tag="gps")
                          for dt in range(DT):
                              nc.tensor.matmul(g_ps[:], wg[:, dt], x2b[:, dt],
                                               start=(dt == 0),
                                               stop=(dt == DT - 1))
                          v_ps = p6ps_v.tile([P, RB], F32, tag="vps")
                          for dt in range(DT):
                              nc.tensor.matmul(v_ps[:], wv[:, dt], x2b[:, dt],
                                               start=(dt == 0),
                                               stop=(dt == DT - 1))
                          gn = p6t.tile([P, RB], F32, tag="gn")
                          nc.vector.tensor_tensor(gn[:], g_ps[:], rms2r[:], MULT)
                          sg = p6t.tile([P, RB], F32, tag="sg")
                          if SIM_SILU:
                              nc.scalar.activation(
                                  sg[:], gn[:],
                                  mybir.ActivationFunctionType.Sigmoid)
                              nc.vector.tensor_tensor(sg[:], sg[:], gn[:], MULT)
                          else:
                              nc.scalar.activation(sg[:], gn[:], SILU)
                          nc.vector.tensor_tensor(gt[:, fi], sg[:], v_ps[:], MULT)
                      for do in range(DT):
                          wm = p6w.tile([P, 11, P], BF16, tag="wm")
                          nc.sync.dma_start(wm[:, :nq], wmlp[:, do, fg0:fg1, :])
                          o_ps = p6ps_o.tile([P, RB], F32, tag="ops6")
                          for fi in range(nq):
                              nc.tensor.matmul(o_ps[:], wm[:, fi], gt[:, fi],
                                               start=(fi == 0),
                                               stop=(fi == nq - 1))
                          if qi == 0:
                              nc.vector.tensor_copy(out_acc[:, do], o_ps[:])
                          elif qi < len(quarters) - 1:
                              nc.vector.tensor_tensor(
                                  out_acc[:, do], o_ps[:], out_acc[:, do], ADD)
                          else:
                              tmp = p6t.tile([P, RB], F32, tag="tmp6")
                              nc.vector.tensor_tensor(
                                  tmp[:], o_ps[:], out_acc[:, do], ADD)
                              fin = p6t.tile([P, RB], F32, tag="fin")
                              nc.vector.tensor_tensor(fin[:], tmp[:], rms2r[:],
                                                      MULT)
                              nc.vector.tensor_tensor(fin[:], fin[:], x2b[:, do],
                                                      ADD)
                              nc.sync.dma_start(outT[do, :, :], fin[:])

    nc.compile()
    return nc


def _rope_tables():
    inv_freq = 1.0 / (ROPE_BASE ** (np.arange(0, DH, 2, dtype=np.float32) / DH))
    t = np.arange(T, dtype=np.float32)
    freqs = np.outer(t, inv_freq)
    emb = np.repeat(freqs, 2, axis=-1)  # [T, DH]
    return np.cos(emb).astype(np.float32), np.sin(emb).astype(np.float32)


def _tile4(w, n_out_tiles, n_in_tiles):
    """[F_out, D_in] -> [P(p of d-tile), F_out/P tiles, D_in/P tiles, P(c of f-tile)].

    Element [p, f, dt, c] = w[f*P + c, dt*P + p].
    """
    Fo, Di = w.shape
    assert Fo == n_out_tiles * P and Di == n_in_tiles * P
    v = w.reshape(n_out_tiles, P, n_in_tiles, P)
    return np.ascontiguousarray(v.transpose(3, 0, 2, 1))


def _prepare_inputs(x, norm1_w, norm2_w, c_attn_w, c_proj_w, c_gate_w, c_val_w,
                    c_mlp_proj_w):
    xf = np.ascontiguousarray(x.reshape(R, D).T)  # [D, R] f32
    xf_bf = xf.astype(NPBF16)
    cos, sin = _rope_tables()
    cosT = np.ascontiguousarray(cos.T)  # [DH, T]
    sinT = np.ascontiguousarray(sin.T)

    # rot-half signed permutation: (P @ q)[d] = -q[d+1] (d even), q[d-1] (d odd)
    rotP = np.zeros((P, P), np.float32)
    for d in range(0, P, 2):
        rotP[d, d + 1] = -1.0
        rotP[d + 1, d] = 1.0
    rotPT = np.ascontiguousarray(rotP.T)

    # additive causal masks for diagonal k-tiles, ST layout [k partition, q col]
    masks = np.zeros((P, QBB, RB), np.float32)
    for di in range(QBB):
        p_idx = np.arange(P)[:, None] + di * P
        c_idx = np.arange(RB)[None, :]
        masks[:, di, :] = np.where(p_idx <= c_idx, 0.0, NEG)

    ones_in = np.ones((P, P), np.float32)
    ones_bf = np.ones((P, P), NPBF16)
    ident_bf = np.eye(P).astype(NPBF16)

    w1 = norm1_w.astype(np.float32)
    w2 = norm2_w.astype(np.float32)
    attn_w = c_attn_w.astype(np.float32) * w1[None, :]     # fold norm1
    gate_w = c_gate_w.astype(np.float32) * w2[None, :]     # fold norm2
    val_w = c_val_w.astype(np.float32) * w2[None, :]

    gate_p = np.zeros((FGP, D), np.float32)
    gate_p[:FG] = gate_w
    val_p = np.zeros((FGP, D), np.float32)
    val_p[:FG] = val_w
    mlp_p = np.zeros((D, FGP), np.float32)
    mlp_p[:, :FG] = c_mlp_proj_w.astype(np.float32)

    # wproj: [p, do, dt, c] -> [p, do, hh, j, c] with dt = 2*j + hh
    wproj_t = _tile4(c_proj_w.astype(np.float32), DT, DT)
    wproj_r = np.ascontiguousarray(
        wproj_t.reshape(P, DT, NCORES, HPC, P).transpose(0, 1, 3, 2, 4)
    ).astype(NPBF16)
    wgate_t = _tile4(gate_p, FGT, DT).astype(NPBF16)
    wval_t = _tile4(val_p, FGT, DT).astype(NPBF16)
    # wmlp: lhsT [fg partition, dout col]: [p, do, fg, c] = mlp_p[do*P+c, fg*P+p]
    wmlp_t = np.ascontiguousarray(
        mlp_p.reshape(DT, P, FGT, P).transpose(3, 0, 2, 1)
    ).astype(NPBF16)

    in_maps = []
    for i in range(NCORES):
        h0, h1 = 2 * i, 2 * i + 1
        rows = []
        for base in (0, D, 2 * D):  # q, k, v row groups of c_attn_w
            rows.extend(range(base + h0 * DH, base + h0 * DH + DH))
            rows.extend(range(base + h1 * DH, base + h1 * DH + DH))
        wsel = attn_w[rows, :]                       # [768, D]
        wqkv_t = _tile4(wsel, NF, DT).astype(NPBF16)
        xres_i = np.ascontiguousarray(
            xf[:, i * RB : (i + 1) * RB].reshape(DT, P, RB)
        )
        in_maps.append({
            "xT": xf_bf,
            "xres": xres_i,
            "wqkv": wqkv_t,
            "wproj": wproj_r,
            "wgate": wgate_t,
            "wval": wval_t,
            "wmlp": wmlp_t,
            "cosT": cosT,
            "sinT": sinT,
            "rotPT": rotPT,
            "masks": masks,
            "ones_in": ones_in,
            "ones_bf": ones_bf,
            "ident_bf": ident_bf,
        })
    return in_maps


_NC_CACHE = None


def _get_program():
    global _NC_CACHE
    if _NC_CACHE is None:
        _NC_CACHE = _build_program()
    return _NC_CACHE


def run(inputs, trace=False):
    """Returns (output [B,T,D], exec_time_ns or None)."""
    in_maps = _prepare_inputs(**inputs)
    nc = _get_program()
    res = run_bass_kernel_spmd(nc, in_maps, list(range(NCORES)), trace=trace)
    cols = []
    for i in range(NCORES):
        o = res.results[i]["outT"]          # [DT, P, RB]
        cols.append(o.reshape(D, RB))
    full_T = np.concatenate(cols, axis=1)   # [D, R]
    out = np.ascontiguousarray(full_T.T).reshape(B, T, D).astype(np.float32)
    return out, res.exec_time_ns


def kernel(**inputs) -> np.ndarray:
    out, _ = run(inputs, trace=False)
    return out
